# revision 7
# baseline (speedup 1.0000x reference)
import importlib.util
import os
import sys

sys.path.insert(0, "/opt/trn_rl_repo")

import numpy as np
from contextlib import ExitStack

N_CORES = 8
N_PTS = 65536
PTS_PER_CORE = N_PTS // N_CORES  # 8192
GROUPS = 4                        # unit-groups, one per 32-partition quadrant
PTS_PER_GROUP = PTS_PER_CORE // GROUPS  # 2048
F = 512                           # points per instruction (free dim)
NT = PTS_PER_GROUP // F           # 4 point-tiles
CH_LIST = ["val", "zx", "zy", "zt", "zxx", "zxy", "zyy", "zxt", "zyt",
           "zxxx", "zxxy", "zxyy", "zyyy"]
CH_IDX = {c: i for i, c in enumerate(CH_LIST)}
BANK = {"val": "A", "zx": "B", "zy": "C", "zt": "D", "zxx": "E", "zxy": "F",
        "zyy": "G", "zxt": "H", "zyt": "A", "zxxx": "B", "zxxy": "C",
        "zxyy": "D", "zyyy": "E"}
ROUND1 = ["val", "zx", "zy", "zt", "zxx", "zxy", "zyy", "zxt"]
ROUND2 = ["zyt", "zxxx", "zxxy", "zxyy", "zyyy"]
PIECES_OF = {
    "val": ["v"], "zx": ["hx"], "zy": ["hy"], "zt": ["ht"],
    "zxx": ["mxx", "nxx"], "zxy": ["mxy", "nxy"], "zyy": ["myy", "nyy"],
    "zxt": ["mxt", "nxt"], "zyt": ["myt", "nyt"],
    "zxxx": ["r1xxx", "r2xxx", "r3xxx"], "zxxy": ["r1xxy", "r2xxy", "r3xxy"],
    "zxyy": ["r1xyy", "r2xyy", "r3xyy"], "zyyy": ["r1yyy", "r2yyy", "r3yyy"],
}

LAST_EXEC_NS = None


def _build_program():
    import concourse.bass as bass
    import concourse.bacc as bacc
    import concourse.tile as tile
    import concourse.mybir as mybir

    f32 = mybir.dt.float32
    AF = mybir.ActivationFunctionType
    ALU = mybir.AluOpType

    nc = bacc.Bacc("TRN2", target_bir_lowering=False, num_devices=N_CORES)
    J1 = nc.declare_dram_parameter("J1", [GROUPS, 3, 13, PTS_PER_GROUP], f32, isOutput=False)
    WB = nc.declare_dram_parameter("WB", [128, 122], f32, isOutput=False)
    BB = nc.declare_dram_parameter("BB", [128, 7], f32, isOutput=False)
    OUT = nc.declare_dram_parameter("OUT", [14, PTS_PER_CORE], f32, isOutput=True)

    with ExitStack() as ctx:
        tc = ctx.enter_context(tile.TileContext(nc))
        const = ctx.enter_context(tc.tile_pool(name="const", bufs=1))
        jets = ctx.enter_context(tc.tile_pool(name="jets", bufs=1))
        pieces = ctx.enter_context(tc.tile_pool(name="pieces", bufs=2))
        work = ctx.enter_context(tc.tile_pool(name="work", bufs=1))
        psum = ctx.enter_context(tc.tile_pool(name="psum", bufs=1, space=bass.MemorySpace.PSUM))

        wb = const.tile([128, 122], f32, name="wb")
        bb = const.tile([128, 7], f32, name="bb")
        nc.sync.dma_start(wb[:], WB[:])
        nc.sync.dma_start(bb[:], BB[:])

        ps = {k: psum.tile([128, F], f32, name=f"ps{k}") for k in "ABCDEFGH"}
        for k in "ABCDEFGH":
            nc.vector.memset(ps[k][:], 0.0)

        def pt(name):
            return pieces.tile([128, F], f32, name=name)

        def wt(name):
            return work.tile([128, F], f32, name=name)

        def emit_round(l, chs, prev, ji):
            din = 3 if l == 2 else 20
            dout = 2 if l == 8 else 20
            off = 20 * (l - 2)
            for chn in chs:
                bank = ps[BANK[chn]]
                if l == 2:
                    srcs_of = lambda g, c=chn: [ji[32 * g:32 * g + 3,
                                                   CH_IDX[c] * F:(CH_IDX[c] + 1) * F]]
                else:
                    srcs_of = lambda g, c=chn: [prev[p][32 * g:32 * g + din, :]
                                                for p in PIECES_OF[c]]
                for g in range(GROUPS):
                    srcs = srcs_of(g)
                    lhsT = wb[32 * g:32 * g + din, off:off + dout]
                    out = bank[32 * g:32 * g + dout, :]
                    # auto tile_position inference rejects base partition 96
                    kw = {"tile_position": (96, 96)} if g == 3 else {}
                    for i, src in enumerate(srcs):
                        nc.tensor.matmul(out, lhsT, src,
                                         start=(i == 0), stop=(i == len(srcs) - 1),
                                         **kw)

        def emit_act(l):
            v = pt("v")
            nc.scalar.activation(v[:], ps["A"][:], AF.Tanh, bias=bb[:, l - 2:l - 1])
            sq = wt("sq")
            nc.scalar.activation(sq[:], v[:], AF.Square)
            c = {}
            for nm, bk in [("x", "B"), ("y", "C"), ("t", "D"),
                           ("xx", "E"), ("xy", "F"), ("yy", "G")]:
                cc = wt("c" + nm)
                nc.scalar.activation(cc[:], ps[bk][:], AF.Copy)
                c[nm] = cc
            return v, sq, c

        def emit_dve(l, v, sq, c, last):
            P = {}
            f1 = wt("f1")
            nc.vector.tensor_scalar(f1[:], sq[:], -1.0, 1.0, ALU.mult, ALU.add)
            # free PSUM banks as early as possible
            nxt = pt("nxt"); nc.vector.tensor_mul(nxt[:], f1[:], ps["H"][:])
            nyt = pt("nyt"); nc.vector.tensor_mul(nyt[:], f1[:], ps["A"][:])
            r3 = {}
            for abc, bk in [("xxx", "B"), ("xxy", "C"), ("xyy", "D"), ("yyy", "E")]:
                r = pt("r3" + abc); nc.vector.tensor_mul(r[:], f1[:], ps[bk][:])
                r3[abc] = r
            f2h = wt("f2h")
            nc.vector.scalar_tensor_tensor(f2h[:], sq[:], 1.0, v[:], ALU.subtract, ALU.mult)
            f3g = wt("f3g")
            nc.vector.scalar_tensor_tensor(f3g[:], sq[:], 1.0 / 3.0, f1[:], ALU.subtract, ALU.mult)
            hx = pt("hx"); nc.vector.tensor_mul(hx[:], f1[:], c["x"][:])
            hy = pt("hy"); nc.vector.tensor_mul(hy[:], f1[:], c["y"][:])
            ht = None
            if not last:
                ht = pt("ht"); nc.vector.tensor_mul(ht[:], f1[:], c["t"][:])
            for ab, (a, b) in [("xx", ("x", "x")), ("xy", ("x", "y")), ("yy", ("y", "y")),
                               ("xt", ("x", "t")), ("yt", ("y", "t"))]:
                pp = wt("p" + ab); nc.vector.tensor_mul(pp[:], c[a][:], c[b][:])
                P[ab] = pp
            m = {}
            for ab in ["xx", "xy", "yy", "xt", "yt"]:
                mm = pt("m" + ab)
                nc.vector.scalar_tensor_tensor(mm[:], P[ab][:], 2.0, f2h[:], ALU.mult, ALU.mult)
                m[ab] = mm
            n = {"xt": nxt, "yt": nyt}
            for ab in ["xx", "xy", "yy"]:
                nn = pt("n" + ab); nc.vector.tensor_mul(nn[:], f1[:], c[ab][:])
                n[ab] = nn
            q = {}
            for qi, (a, b) in [("1", ("xx", "x")), ("2", ("xx", "y")), ("3", ("xy", "x")),
                               ("4", ("xy", "y")), ("5", ("yy", "x")), ("6", ("yy", "y"))]:
                qq = wt("q" + qi); nc.vector.tensor_mul(qq[:], c[a][:], c[b][:])
                q[qi] = qq
            sxxy = wt("sxxy")
            nc.vector.scalar_tensor_tensor(sxxy[:], q["3"][:], 2.0, q["2"][:], ALU.mult, ALU.add)
            sxyy = wt("sxyy")
            nc.vector.scalar_tensor_tensor(sxyy[:], q["4"][:], 2.0, q["5"][:], ALU.mult, ALU.add)
            T = {}
            for abc, (pab, a) in [("xxx", ("xx", "x")), ("xxy", ("xx", "y")),
                                  ("xyy", ("yy", "x")), ("yyy", ("yy", "y"))]:
                tt = wt("t" + abc); nc.vector.tensor_mul(tt[:], P[pab][:], c[a][:])
                T[abc] = tt
            r1 = {}
            for abc in ["xxx", "xxy", "xyy", "yyy"]:
                rr = pt("r1" + abc)
                nc.vector.scalar_tensor_tensor(rr[:], T[abc][:], 6.0, f3g[:], ALU.mult, ALU.mult)
                r1[abc] = rr
            r2 = {}
            for abc, (src, k) in [("xxx", (q["1"], 6.0)), ("xxy", (sxxy, 2.0)),
                                  ("xyy", (sxyy, 2.0)), ("yyy", (q["6"], 6.0))]:
                rr = pt("r2" + abc)
                nc.vector.scalar_tensor_tensor(rr[:], src[:], k, f2h[:], ALU.mult, ALU.mult)
                r2[abc] = rr

            if not last:
                out = {"v": v, "hx": hx, "hy": hy, "ht": ht}
                for ab in ["xx", "xy", "yy", "xt", "yt"]:
                    out["m" + ab] = m[ab]
                    out["n" + ab] = n[ab]
                for abc in ["xxx", "xxy", "xyy", "yyy"]:
                    out["r1" + abc] = r1[abc]
                    out["r2" + abc] = r2[abc]
                    out["r3" + abc] = r3[abc]
                return out
            # last layer: fold pieces into final jets (in-place adds)
            for ab in ["xx", "xy", "yy", "xt", "yt"]:
                nc.vector.tensor_add(m[ab][:], m[ab][:], n[ab][:])
            for abc in ["xxx", "xxy", "xyy", "yyy"]:
                nc.vector.tensor_add(r1[abc][:], r1[abc][:], r2[abc][:])
                nc.vector.tensor_add(r1[abc][:], r1[abc][:], r3[abc][:])
            return {
                0: (hx, 0), 1: (hy, 0),
                2: (m["xx"], 0), 3: (m["xy"], 0), 4: (m["yy"], 0),
                5: (m["xt"], 0), 6: (m["yt"], 0),
                7: (r1["xxx"], 0), 8: (r1["xxy"], 0), 9: (r1["xyy"], 0), 10: (r1["yyy"], 0),
                11: (v, 1), 12: (hx, 1), 13: (hy, 1),
            }

        for t in range(NT):
            ji = jets.tile([128, 13 * F], f32, name="ji")
            for g in range(GROUPS):
                nc.sync.dma_start(ji[32 * g:32 * g + 3, :], J1[g, :, :, bass.ts(t, F)])
            prev = None
            for l in range(2, 9):
                emit_round(l, ROUND1, prev, ji)
                v, sq, c = emit_act(l)
                emit_round(l, ROUND2, prev, ji)
                prev = emit_dve(l, v, sq, c, last=(l == 8))
            for row, (tl, unit) in prev.items():
                for g in range(GROUPS):
                    col0 = PTS_PER_GROUP * g + F * t
                    nc.scalar.dma_start(OUT[row:row + 1, col0:col0 + F],
                                        tl[32 * g + unit:32 * g + unit + 1, :])

    nc.finalize()
    return nc


_NC = None


def _get_nc():
    global _NC
    if _NC is None:
        _NC = _build_program()
    return _NC


def _host_pack(inputs):
    X32 = np.asarray(inputs["X"], dtype=np.float32)
    X = X32.astype(np.float64)
    Ws = [np.asarray(inputs[f"W{i}"], dtype=np.float64) for i in range(1, 9)]
    bs = [np.asarray(inputs[f"b{i}"], dtype=np.float64) for i in range(1, 9)]

    lb = float(X32[:, 0].min())
    ub = float(X32[:, 0].max())
    s = 2.0 / (ub - lb)
    cshift = -2.0 * lb / (ub - lb) - 1.0
    W1e = s * Ws[0]                      # [3,3]
    b1e = bs[0] + cshift * Ws[0].sum(axis=0)

    Z1 = X @ W1e + b1e                   # [N,3]
    y = np.tanh(Z1)
    sq = y * y
    f1 = 1.0 - sq
    f2 = -2.0 * y * f1
    f3 = f1 * (6.0 * sq - 2.0)
    ux, uy, ut = W1e[0], W1e[1], W1e[2]  # each [3]

    jets = np.empty((13, N_PTS, 3), dtype=np.float64)
    jets[0] = y
    jets[1] = f1 * ux
    jets[2] = f1 * uy
    jets[3] = f1 * ut
    jets[4] = f2 * (ux * ux)
    jets[5] = f2 * (ux * uy)
    jets[6] = f2 * (uy * uy)
    jets[7] = f2 * (ux * ut)
    jets[8] = f2 * (uy * ut)
    jets[9] = f3 * (ux * ux * ux)
    jets[10] = f3 * (ux * ux * uy)
    jets[11] = f3 * (ux * uy * uy)
    jets[12] = f3 * (uy * uy * uy)
    jets = jets.astype(np.float32)

    WB = np.zeros((128, 122), dtype=np.float32)
    BBp = np.zeros((128, 7), dtype=np.float32)
    for l in range(2, 9):
        W = Ws[l - 1].astype(np.float32)
        b = bs[l - 1].astype(np.float32)
        din, dout = W.shape
        off = 20 * (l - 2)
        for g in range(GROUPS):
            WB[32 * g:32 * g + din, off:off + dout] = W
            BBp[32 * g:32 * g + dout, l - 2] = b

    in_maps = []
    for k in range(N_CORES):
        A = jets[:, PTS_PER_CORE * k:PTS_PER_CORE * (k + 1), :]      # [13,8192,3]
        B = A.reshape(13, GROUPS, PTS_PER_GROUP, 3)
        J1k = np.ascontiguousarray(B.transpose(1, 3, 0, 2), dtype=np.float32)
        in_maps.append({"J1": J1k, "WB": WB, "BB": BBp})
    return in_maps


def kernel(**inputs):
    global LAST_EXEC_NS
    from concourse.bass_utils import run_bass_kernel_spmd

    nc = _get_nc()
    in_maps = _host_pack(inputs)
    trace = bool(os.environ.get("BASS_KERNEL_TRACE"))
    if trace and importlib.util.find_spec("antenv.axon_hooks") is None:
        trace = False
    kw = {}
    if trace:
        kw["trace"] = True
        td = os.environ.get("BASS_KERNEL_TRACE_DIR")
        if td:
            kw["tmpdir"] = td
    res = run_bass_kernel_spmd(nc, in_maps, list(range(N_CORES)), **kw)
    LAST_EXEC_NS = res.exec_time_ns
    O = np.concatenate([np.asarray(res.results[k]["OUT"]) for k in range(N_CORES)],
                       axis=1).astype(np.float32)  # [14, 65536]

    lam1 = np.float32(np.asarray(inputs["lam1"]).reshape(-1)[0])
    lam2 = np.float32(np.asarray(inputs["lam2"]).reshape(-1)[0])
    u = O[1].copy()
    vv = (-O[0]).astype(np.float32)
    p = O[11].copy()
    f_u = O[6] + lam1 * (O[1] * O[3] - O[0] * O[4]) + O[12] - lam2 * (O[8] + O[10])
    f_v = -O[5] + lam1 * (O[0] * O[3] - O[1] * O[2]) + O[13] + lam2 * (O[7] + O[9])
    return (u, vv, p[:, None].copy(),
            f_u.astype(np.float32), f_v.astype(np.float32))


# revision 26
# speedup vs baseline: 1.3017x; 1.3017x over previous
import importlib.util
import os
import sys

sys.path.insert(0, "/opt/trn_rl_repo")

import numpy as np
from contextlib import ExitStack

N_CORES = 8
N_PTS = 65536
PTS_PER_CORE = N_PTS // N_CORES  # 8192
GROUPS = 6                        # unit-groups of 20 partitions (120/128 used)
GS = 20                           # partition stride per group
F = 512                           # points per instruction (free dim)
NT = 3                            # super-tiles; capacity 6*512*3 = 9216 (pad 1024)
PTS_PER_GROUP = NT * F            # 1536
PAD_PTS = GROUPS * PTS_PER_GROUP  # 9216
CH_LIST = ["val", "zx", "zy", "zt", "zxx", "zxy", "zyy", "zxt", "zyt",
           "zxxx", "zxxy", "zxyy", "zyyy"]
CH_IDX = {c: i for i, c in enumerate(CH_LIST)}
BANK = {"val": "A", "zx": "B", "zy": "C", "zt": "D", "zxx": "E", "zxy": "F",
        "zyy": "G", "zxt": "H", "zyt": "A", "zxxx": "B", "zxxy": "C",
        "zxyy": "D", "zyyy": "E"}
ROUND1 = ["val", "zx", "zy", "zt", "zxx", "zxy", "zyy", "zxt"]
ROUND2 = ["zyt", "zxxx", "zxxy", "zxyy", "zyyy"]
PIECES_OF = {
    "val": ["v"], "zx": ["hx"], "zy": ["hy"], "zt": ["ht"],
    "zxx": ["mxx", "nxx"], "zxy": ["mxy", "nxy"], "zyy": ["myy", "nyy"],
    "zxt": ["mnxt"], "zyt": ["mnyt"],
    "zxxx": ["r1xxx", "r2xxx", "r3xxx"], "zxxy": ["r1xxy", "r2xxy", "r3xxy"],
    "zxyy": ["r1xyy", "r2xyy", "r3xyy"], "zyyy": ["r1yyy", "r2yyy", "r3yyy"],
}

LAST_EXEC_NS = None


def _build_program():
    import concourse.bass as bass
    import concourse.bacc as bacc
    import concourse.tile as tile
    import concourse.mybir as mybir

    f32 = mybir.dt.float32
    AF = mybir.ActivationFunctionType
    ALU = mybir.AluOpType

    nc = bacc.Bacc("TRN2", target_bir_lowering=False, num_devices=N_CORES)
    J1 = nc.declare_dram_parameter("J1", [GROUPS, 3, 13, PTS_PER_GROUP], f32, isOutput=False)
    WBD = nc.declare_dram_parameter("WBD", [128, 7 * 128], f32, isOutput=False)
    BB = nc.declare_dram_parameter("BB", [128, 7], f32, isOutput=False)
    OUT = nc.declare_dram_parameter("OUT", [14, PAD_PTS], f32, isOutput=True)

    with ExitStack() as ctx:
        tc = ctx.enter_context(tile.TileContext(nc))
        const = ctx.enter_context(tc.tile_pool(name="const", bufs=1))
        jets = ctx.enter_context(tc.tile_pool(name="jets", bufs=1))
        pieces = ctx.enter_context(tc.tile_pool(name="pieces", bufs=2))
        work = ctx.enter_context(tc.tile_pool(name="work", bufs=1))
        psum = ctx.enter_context(tc.tile_pool(name="psum", bufs=1, space=bass.MemorySpace.PSUM))

        wbd = const.tile([128, 7 * 128], f32, name="wbd")
        bb = const.tile([128, 7], f32, name="bb")
        nc.sync.dma_start(wbd[:], WBD[:])
        nc.sync.dma_start(bb[:], BB[:])

        ps = {k: psum.tile([128, F], f32, name=f"ps{k}") for k in "ABCDEFGH"}

        def pt(name):
            return pieces.tile([128, F], f32, name=name)

        def wt(name):
            return work.tile([128, F], f32, name=name)

        def emit_round(l, chs, prev, ji):
            off = 128 * (l - 2)
            lhsT = wbd[:, off:off + 128]
            for chn in chs:
                bank = ps[BANK[chn]]
                if l == 2:
                    srcs = [ji[:, CH_IDX[chn] * F:(CH_IDX[chn] + 1) * F]]
                else:
                    srcs = [prev[p][:, :] for p in PIECES_OF[chn]]
                for i, src in enumerate(srcs):
                    nc.tensor.matmul(bank[:, :], lhsT, src,
                                     start=(i == 0), stop=(i == len(srcs) - 1))

        def emit_act(l):
            v = pt("v")
            nc.scalar.activation(v[:], ps["A"][:], AF.Tanh, bias=bb[:, l - 2:l - 1])
            sq = wt("sq")
            nc.scalar.activation(sq[:], v[:], AF.Square)
            c = {}
            for nm, bk in [("x", "B"), ("y", "C"), ("t", "D"),
                           ("xx", "E"), ("xy", "F"), ("yy", "G")]:
                cc = wt("c" + nm)
                nc.scalar.activation(cc[:], ps[bk][:], AF.Copy)
                c[nm] = cc
            return v, sq, c

        def emit_dve(l, v, sq, c, last):
            P = {}
            f1 = wt("f1")
            nc.vector.tensor_scalar(f1[:], sq[:], -1.0, 1.0, ALU.mult, ALU.add)
            # free PSUM banks as early as possible
            nxt = wt("nxt"); nc.vector.tensor_mul(nxt[:], f1[:], ps["H"][:])
            nyt = wt("nyt"); nc.vector.tensor_mul(nyt[:], f1[:], ps["A"][:])
            r3 = {}
            for abc, bk in [("xxx", "B"), ("xxy", "C"), ("xyy", "D"), ("yyy", "E")]:
                r = pt("r3" + abc); nc.vector.tensor_mul(r[:], f1[:], ps[bk][:])
                r3[abc] = r
            f2h = wt("f2h")
            nc.vector.scalar_tensor_tensor(f2h[:], sq[:], 1.0, v[:], ALU.subtract, ALU.mult)
            f3g = wt("f3g")
            nc.vector.scalar_tensor_tensor(f3g[:], sq[:], 1.0 / 3.0, f1[:], ALU.subtract, ALU.mult)
            hx = pt("hx"); nc.vector.tensor_mul(hx[:], f1[:], c["x"][:])
            hy = pt("hy"); nc.vector.tensor_mul(hy[:], f1[:], c["y"][:])
            ht = None
            if not last:
                ht = pt("ht"); nc.vector.tensor_mul(ht[:], f1[:], c["t"][:])
            for ab, (a, b) in [("xx", ("x", "x")), ("xy", ("x", "y")), ("yy", ("y", "y")),
                               ("xt", ("x", "t")), ("yt", ("y", "t"))]:
                pp = wt("p" + ab); nc.gpsimd.tensor_mul(pp[:], c[a][:], c[b][:])
                P[ab] = pp
            m = {}
            for ab in ["xx", "xy", "yy", "xt", "yt"]:
                mm = pt("m" + ab) if ab in ("xx", "xy", "yy") else wt("m" + ab)
                nc.vector.scalar_tensor_tensor(mm[:], P[ab][:], 2.0, f2h[:], ALU.mult, ALU.mult)
                m[ab] = mm
            n = {"xt": nxt, "yt": nyt}
            for ab, eng in [("xx", nc.gpsimd), ("xy", nc.gpsimd), ("yy", nc.vector)]:
                nn = pt("n" + ab); eng.tensor_mul(nn[:], f1[:], c[ab][:])
                n[ab] = nn
            q = {}
            for qi, (a, b) in [("1", ("xx", "x")), ("2", ("xx", "y")), ("3", ("xy", "x")),
                               ("4", ("xy", "y")), ("5", ("yy", "x")), ("6", ("yy", "y"))]:
                qq = wt("q" + qi); nc.gpsimd.tensor_mul(qq[:], c[a][:], c[b][:])
                q[qi] = qq
            sxxy = wt("sxxy")
            nc.vector.scalar_tensor_tensor(sxxy[:], q["3"][:], 2.0, q["2"][:], ALU.mult, ALU.add)
            sxyy = wt("sxyy")
            nc.vector.scalar_tensor_tensor(sxyy[:], q["4"][:], 2.0, q["5"][:], ALU.mult, ALU.add)
            T = {}
            for abc, (pab, a) in [("xxx", ("xx", "x")), ("xxy", ("xx", "y")),
                                  ("xyy", ("yy", "x")), ("yyy", ("yy", "y"))]:
                tt = wt("t" + abc); nc.gpsimd.tensor_mul(tt[:], P[pab][:], c[a][:])
                T[abc] = tt
            r1 = {}
            for abc in ["xxx", "xxy", "xyy", "yyy"]:
                rr = pt("r1" + abc)
                nc.vector.scalar_tensor_tensor(rr[:], T[abc][:], 6.0, f3g[:], ALU.mult, ALU.mult)
                r1[abc] = rr
            r2 = {}
            for abc, (src, k) in [("xxx", (q["1"], 6.0)), ("xxy", (sxxy, 2.0)),
                                  ("xyy", (sxyy, 2.0)), ("yyy", (q["6"], 6.0))]:
                rr = pt("r2" + abc)
                nc.vector.scalar_tensor_tensor(rr[:], src[:], k, f2h[:], ALU.mult, ALU.mult)
                r2[abc] = rr

            if not last:
                out = {"v": v, "hx": hx, "hy": hy, "ht": ht}
                for ab in ["xt", "yt"]:
                    z = pt("mn" + ab)
                    nc.vector.tensor_add(z[:], m[ab][:], n[ab][:])
                    out["mn" + ab] = z
                for ab in ["xx", "xy", "yy"]:
                    out["m" + ab] = m[ab]
                    out["n" + ab] = n[ab]
                for abc in ["xxx", "xxy", "xyy", "yyy"]:
                    out["r1" + abc] = r1[abc]
                    out["r2" + abc] = r2[abc]
                    out["r3" + abc] = r3[abc]
                return out
            # last layer: fold pieces into final jets (in-place adds)
            for ab in ["xx", "xy", "yy", "xt", "yt"]:
                nc.vector.tensor_add(m[ab][:], m[ab][:], n[ab][:])
            for abc in ["xxx", "xxy", "xyy", "yyy"]:
                nc.vector.tensor_add(r1[abc][:], r1[abc][:], r2[abc][:])
                nc.vector.tensor_add(r1[abc][:], r1[abc][:], r3[abc][:])
            return {
                0: (hx, 0), 1: (hy, 0),
                2: (m["xx"], 0), 3: (m["xy"], 0), 4: (m["yy"], 0),
                5: (m["xt"], 0), 6: (m["yt"], 0),
                7: (r1["xxx"], 0), 8: (r1["xxy"], 0), 9: (r1["xyy"], 0), 10: (r1["yyy"], 0),
                11: (v, 1), 12: (hx, 1), 13: (hy, 1),
            }

        ji = jets.tile([128, 13 * F], f32, name="ji")
        nc.vector.memset(ji[:], 0.0)

        def load_ji(t):
            for g in range(GROUPS):
                nc.sync.dma_start(ji[GS * g:GS * g + 3, :], J1[g, :, :, bass.ts(t, F)])

        load_ji(0)
        for t in range(NT):
            prev = None
            for l in range(2, 9):
                emit_round(l, ROUND1, prev, ji)
                v, sq, c = emit_act(l)
                emit_round(l, ROUND2, prev, ji)
                if l == 2 and t + 1 < NT:
                    # queue next super-tile's input load ahead of this tile's
                    # output DMAs so it isn't stuck behind them on the DMA HW
                    load_ji(t + 1)
                prev = emit_dve(l, v, sq, c, last=(l == 8))
            for row, (tl, unit) in prev.items():
                for g in range(GROUPS):
                    col0 = GROUPS * F * t + F * g
                    nc.scalar.dma_start(OUT[row:row + 1, col0:col0 + F],
                                        tl[GS * g + unit:GS * g + unit + 1, :])

    nc.finalize()
    return nc


_NC = None


def _get_nc():
    global _NC
    if _NC is None:
        _NC = _build_program()
    return _NC


def _host_pack(inputs):
    X32 = np.asarray(inputs["X"], dtype=np.float32)
    X = X32.astype(np.float64)
    Ws = [np.asarray(inputs[f"W{i}"], dtype=np.float64) for i in range(1, 9)]
    bs = [np.asarray(inputs[f"b{i}"], dtype=np.float64) for i in range(1, 9)]

    lb = float(X32[:, 0].min())
    ub = float(X32[:, 0].max())
    s = 2.0 / (ub - lb)
    cshift = -2.0 * lb / (ub - lb) - 1.0
    W1e = s * Ws[0]                      # [3,3]
    b1e = bs[0] + cshift * Ws[0].sum(axis=0)

    Z1 = X @ W1e + b1e                   # [N,3]
    y = np.tanh(Z1)
    sq = y * y
    f1 = 1.0 - sq
    f2 = -2.0 * y * f1
    f3 = f1 * (6.0 * sq - 2.0)
    ux, uy, ut = W1e[0], W1e[1], W1e[2]  # each [3]

    jets = np.empty((13, N_PTS, 3), dtype=np.float64)
    jets[0] = y
    jets[1] = f1 * ux
    jets[2] = f1 * uy
    jets[3] = f1 * ut
    jets[4] = f2 * (ux * ux)
    jets[5] = f2 * (ux * uy)
    jets[6] = f2 * (uy * uy)
    jets[7] = f2 * (ux * ut)
    jets[8] = f2 * (uy * ut)
    jets[9] = f3 * (ux * ux * ux)
    jets[10] = f3 * (ux * ux * uy)
    jets[11] = f3 * (ux * uy * uy)
    jets[12] = f3 * (uy * uy * uy)
    jets = jets.astype(np.float32)

    WBD = np.zeros((128, 7 * 128), dtype=np.float32)
    BBp = np.zeros((128, 7), dtype=np.float32)
    for l in range(2, 9):
        W = Ws[l - 1].astype(np.float32)
        b = bs[l - 1].astype(np.float32)
        din, dout = W.shape
        off = 128 * (l - 2)
        for g in range(GROUPS):
            WBD[GS * g:GS * g + din, off + GS * g:off + GS * g + dout] = W
            BBp[GS * g:GS * g + dout, l - 2] = b

    in_maps = []
    for k in range(N_CORES):
        A = jets[:, PTS_PER_CORE * k:PTS_PER_CORE * (k + 1), :]      # [13,8192,3]
        Ap = np.zeros((13, PAD_PTS, 3), dtype=np.float32)
        Ap[:, :PTS_PER_CORE] = A
        B = Ap.reshape(13, NT, GROUPS, F, 3)
        J1k = np.ascontiguousarray(
            B.transpose(2, 4, 0, 1, 3).reshape(GROUPS, 3, 13, NT * F))
        in_maps.append({"J1": J1k, "WBD": WBD, "BB": BBp})
    return in_maps


def kernel(**inputs):
    global LAST_EXEC_NS
    from concourse.bass_utils import run_bass_kernel_spmd

    nc = _get_nc()
    in_maps = _host_pack(inputs)
    trace = bool(os.environ.get("BASS_KERNEL_TRACE"))
    if trace and importlib.util.find_spec("antenv.axon_hooks") is None:
        trace = False
    kw = {}
    if trace:
        kw["trace"] = True
        td = os.environ.get("BASS_KERNEL_TRACE_DIR")
        if td:
            kw["tmpdir"] = td
    res = run_bass_kernel_spmd(nc, in_maps, list(range(N_CORES)), **kw)
    LAST_EXEC_NS = res.exec_time_ns
    O = np.concatenate(
        [np.asarray(res.results[k]["OUT"])[:, :PTS_PER_CORE] for k in range(N_CORES)],
        axis=1).astype(np.float32)  # [14, 65536]

    lam1 = np.float32(np.asarray(inputs["lam1"]).reshape(-1)[0])
    lam2 = np.float32(np.asarray(inputs["lam2"]).reshape(-1)[0])
    u = O[1].copy()
    vv = (-O[0]).astype(np.float32)
    p = O[11].copy()
    f_u = O[6] + lam1 * (O[1] * O[3] - O[0] * O[4]) + O[12] - lam2 * (O[8] + O[10])
    f_v = -O[5] + lam1 * (O[0] * O[3] - O[1] * O[2]) + O[13] + lam2 * (O[7] + O[9])
    return (u, vv, p[:, None].copy(),
            f_u.astype(np.float32), f_v.astype(np.float32))


# revision 35
# speedup vs baseline: 1.7452x; 1.3407x over previous
import importlib.util
import os
import sys

sys.path.insert(0, "/opt/trn_rl_repo")

import numpy as np
from contextlib import ExitStack

N_CORES = 8
N_PTS = 65536
PTS_PER_CORE = N_PTS // N_CORES  # 8192
GROUPS = 6                        # unit-groups of 20 partitions (120/128 used)
GS = 20                           # partition stride per group
F = 512                           # points per instruction (free dim)
NT = 3                            # super-tiles; capacity 6*512*3 = 9216 (pad 1024)
PTS_PER_GROUP = NT * F            # 1536
PAD_PTS = GROUPS * PTS_PER_GROUP  # 9216
CH_LIST = ["val", "zx", "zy", "zt", "zxx", "zxy", "zyy", "zxt", "zyt",
           "zxxx", "zxxy", "zxyy", "zyyy"]
CH_IDX = {c: i for i, c in enumerate(CH_LIST)}
BANK = {"val": "A", "zx": "B", "zy": "C", "zt": "D", "zxx": "E", "zxy": "F",
        "zyy": "G", "zxt": "H", "zyt": "A", "zxxx": "B", "zxxy": "C",
        "zxyy": "D", "zyyy": "E"}
ROUND1 = ["val", "zx", "zy", "zt", "zxx", "zxy", "zyy", "zxt"]
ROUND2 = ["zyt", "zxxx", "zxxy", "zxyy", "zyyy"]
PIECES_OF = {
    "val": ["v"], "zx": ["hx"], "zy": ["hy"], "zt": ["ht"],
    "zxx": ["mxx", "nxx"], "zxy": ["mxy", "nxy"], "zyy": ["myy", "nyy"],
    "zxt": ["mnxt"], "zyt": ["mnyt"],
    "zxxx": ["r1xxx", "r2xxx", "r3xxx"], "zxxy": ["r1xxy", "r2xxy", "r3xxy"],
    "zxyy": ["r1xyy", "r2xyy", "r3xyy"], "zyyy": ["r1yyy", "r2yyy", "r3yyy"],
}
# output row -> (final-layer piece tile, unit offset within group)
ROWS = [("hx", 0), ("hy", 0), ("mxx", 0), ("mxy", 0), ("myy", 0),
        ("mnxt", 0), ("mnyt", 0), ("r1xxx", 0), ("r1xxy", 0), ("r1xyy", 0),
        ("r1yyy", 0), ("v", 1), ("hx", 1), ("hy", 1)]
TILE_ORDER = ["hx", "hy", "mxx", "mxy", "myy", "mnxt", "mnyt",
              "r1xxx", "r1xxy", "r1xyy", "r1yyy", "v"]

LAST_EXEC_NS = None


def _build_program():
    import concourse.bass as bass
    import concourse.bacc as bacc
    import concourse.tile as tile
    import concourse.mybir as mybir

    f32 = mybir.dt.float32
    AF = mybir.ActivationFunctionType
    ALU = mybir.AluOpType

    nc = bacc.Bacc("TRN2", target_bir_lowering=False, num_devices=N_CORES)
    J1 = nc.declare_dram_parameter("J1", [GROUPS, 3, 13, PTS_PER_GROUP], f32, isOutput=False)
    WBD = nc.declare_dram_parameter("WBD", [128, 7 * 128], f32, isOutput=False)
    BB = nc.declare_dram_parameter("BB", [128, 7], f32, isOutput=False)
    SEL = nc.declare_dram_parameter("SEL", [128, 12 * 84], f32, isOutput=False)
    OUT = nc.declare_dram_parameter("OUT", [14, NT, GROUPS, F], f32, isOutput=True)

    with ExitStack() as ctx:
        tc = ctx.enter_context(tile.TileContext(nc))
        const = ctx.enter_context(tc.tile_pool(name="const", bufs=1))
        jets = ctx.enter_context(tc.tile_pool(name="jets", bufs=1))
        pieces = ctx.enter_context(tc.tile_pool(name="pieces", bufs=2))
        work = ctx.enter_context(tc.tile_pool(name="work", bufs=1))
        psum = ctx.enter_context(tc.tile_pool(name="psum", bufs=1, space=bass.MemorySpace.PSUM))

        wbd = const.tile([128, 7 * 128], f32, name="wbd")
        bb = const.tile([128, 7], f32, name="bb")
        sel = const.tile([128, 12 * 84], f32, name="sel")
        nc.sync.dma_start(wbd[:], WBD[:])
        nc.sync.dma_start(bb[:], BB[:])
        nc.sync.dma_start(sel[:], SEL[:])

        ps = {k: psum.tile([128, F], f32, name=f"ps{k}") for k in "ABCDEFGH"}

        def pt(name):
            return pieces.tile([128, F], f32, name=name)

        def wt(name):
            return work.tile([128, F], f32, name=name)

        def emit_round(l, chs, prev, ji):
            off = 128 * (l - 2)
            lhsT = wbd[:, off:off + 128]
            for chn in chs:
                bank = ps[BANK[chn]]
                if l == 2:
                    srcs = [ji[:, CH_IDX[chn] * F:(CH_IDX[chn] + 1) * F]]
                else:
                    srcs = [prev[p][:, :] for p in PIECES_OF[chn]]
                for i, src in enumerate(srcs):
                    nc.tensor.matmul(bank[:, :], lhsT, src,
                                     start=(i == 0), stop=(i == len(srcs) - 1))

        def emit_act(l):
            v = pt("v")
            nc.scalar.activation(v[:], ps["A"][:], AF.Tanh, bias=bb[:, l - 2:l - 1])
            sq = wt("sq")
            nc.scalar.activation(sq[:], v[:], AF.Square)
            c = {}
            for nm, bk in [("x", "B"), ("y", "C"), ("t", "D"),
                           ("xx", "E"), ("xy", "F"), ("yy", "G")]:
                cc = wt("c" + nm)
                nc.scalar.activation(cc[:], ps[bk][:], AF.Copy)
                c[nm] = cc
            return v, sq, c

        def emit_dve(l, v, sq, c, last):
            P = {}
            f1 = wt("f1")
            nc.vector.tensor_scalar(f1[:], sq[:], -1.0, 1.0, ALU.mult, ALU.add)
            # free PSUM banks as early as possible
            nxt = wt("nxt"); nc.vector.tensor_mul(nxt[:], f1[:], ps["H"][:])
            nyt = wt("nyt"); nc.vector.tensor_mul(nyt[:], f1[:], ps["A"][:])
            r3 = {}
            for abc, bk in [("xxx", "B"), ("xxy", "C"), ("xyy", "D"), ("yyy", "E")]:
                r = pt("r3" + abc); nc.vector.tensor_mul(r[:], f1[:], ps[bk][:])
                r3[abc] = r
            f2h = wt("f2h")
            nc.vector.scalar_tensor_tensor(f2h[:], sq[:], 1.0, v[:], ALU.subtract, ALU.mult)
            f3g = wt("f3g")
            nc.vector.scalar_tensor_tensor(f3g[:], sq[:], 1.0 / 3.0, f1[:], ALU.subtract, ALU.mult)
            hx = pt("hx"); nc.vector.tensor_mul(hx[:], f1[:], c["x"][:])
            hy = pt("hy"); nc.vector.tensor_mul(hy[:], f1[:], c["y"][:])
            ht = None
            if not last:
                ht = pt("ht"); nc.vector.tensor_mul(ht[:], f1[:], c["t"][:])
            for ab, (a, b) in [("xx", ("x", "x")), ("xy", ("x", "y")), ("yy", ("y", "y")),
                               ("xt", ("x", "t")), ("yt", ("y", "t"))]:
                pp = wt("p" + ab); nc.gpsimd.tensor_mul(pp[:], c[a][:], c[b][:])
                P[ab] = pp
            m = {}
            for ab in ["xx", "xy", "yy", "xt", "yt"]:
                if ab in ("xx", "xy", "yy"):
                    mm = pt("m" + ab)
                elif last:
                    # must outlive this tile (read by deferred compaction)
                    mm = pt("mn" + ab)
                else:
                    mm = wt("m" + ab)
                nc.vector.scalar_tensor_tensor(mm[:], P[ab][:], 2.0, f2h[:], ALU.mult, ALU.mult)
                m[ab] = mm
            n = {"xt": nxt, "yt": nyt}
            for ab, eng in [("xx", nc.gpsimd), ("xy", nc.gpsimd), ("yy", nc.vector)]:
                nn = pt("n" + ab); eng.tensor_mul(nn[:], f1[:], c[ab][:])
                n[ab] = nn
            q = {}
            for qi, (a, b) in [("1", ("xx", "x")), ("2", ("xx", "y")), ("3", ("xy", "x")),
                               ("4", ("xy", "y")), ("5", ("yy", "x")), ("6", ("yy", "y"))]:
                qq = wt("q" + qi); nc.gpsimd.tensor_mul(qq[:], c[a][:], c[b][:])
                q[qi] = qq
            sxxy = wt("sxxy")
            nc.vector.scalar_tensor_tensor(sxxy[:], q["3"][:], 2.0, q["2"][:], ALU.mult, ALU.add)
            sxyy = wt("sxyy")
            nc.vector.scalar_tensor_tensor(sxyy[:], q["4"][:], 2.0, q["5"][:], ALU.mult, ALU.add)
            T = {}
            for abc, (pab, a) in [("xxx", ("xx", "x")), ("xxy", ("xx", "y")),
                                  ("xyy", ("yy", "x")), ("yyy", ("yy", "y"))]:
                tt = wt("t" + abc); nc.gpsimd.tensor_mul(tt[:], P[pab][:], c[a][:])
                T[abc] = tt
            r1 = {}
            for abc in ["xxx", "xxy", "xyy", "yyy"]:
                rr = pt("r1" + abc)
                nc.vector.scalar_tensor_tensor(rr[:], T[abc][:], 6.0, f3g[:], ALU.mult, ALU.mult)
                r1[abc] = rr
            r2 = {}
            for abc, (src, k) in [("xxx", (q["1"], 6.0)), ("xxy", (sxxy, 2.0)),
                                  ("xyy", (sxyy, 2.0)), ("yyy", (q["6"], 6.0))]:
                rr = pt("r2" + abc)
                nc.vector.scalar_tensor_tensor(rr[:], src[:], k, f2h[:], ALU.mult, ALU.mult)
                r2[abc] = rr

            if not last:
                out = {"v": v, "hx": hx, "hy": hy, "ht": ht}
                for ab in ["xt", "yt"]:
                    z = pt("mn" + ab)
                    nc.vector.tensor_add(z[:], m[ab][:], n[ab][:])
                    out["mn" + ab] = z
                for ab in ["xx", "xy", "yy"]:
                    out["m" + ab] = m[ab]
                    out["n" + ab] = n[ab]
                for abc in ["xxx", "xxy", "xyy", "yyy"]:
                    out["r1" + abc] = r1[abc]
                    out["r2" + abc] = r2[abc]
                    out["r3" + abc] = r3[abc]
                return out
            # last layer: fold pieces into final jets (in-place adds)
            for ab in ["xx", "xy", "yy", "xt", "yt"]:
                nc.vector.tensor_add(m[ab][:], m[ab][:], n[ab][:])
            for abc in ["xxx", "xxy", "xyy", "yyy"]:
                nc.vector.tensor_add(r1[abc][:], r1[abc][:], r2[abc][:])
                nc.vector.tensor_add(r1[abc][:], r1[abc][:], r3[abc][:])
            return {"hx": hx, "hy": hy, "mxx": m["xx"], "mxy": m["xy"],
                    "myy": m["yy"], "mnxt": m["xt"], "mnyt": m["yt"],
                    "r1xxx": r1["xxx"], "r1xxy": r1["xxy"], "r1xyy": r1["xyy"],
                    "r1yyy": r1["yyy"], "v": v}

        ji = jets.tile([128, 13 * F], f32, name="ji")
        nc.vector.memset(ji[:], 0.0)

        def load_ji(t):
            for g in range(GROUPS):
                nc.sync.dma_start(ji[GS * g:GS * g + 3, :], J1[g, :, :, bass.ts(t, F)])

        def emit_compact(fin, t):
            # gather the 14 output rows x 6 groups into partitions 6*row+g of
            # one PSUM bank via selection matmuls, then 1 copy + 1 DMA descriptor
            bank = ps["H"]
            for i, nm in enumerate(TILE_ORDER):
                nc.tensor.matmul(bank[0:84, :], sel[:, 84 * i:84 * (i + 1)],
                                 fin[nm][:, :], start=(i == 0), stop=(i == 11))
            cons = wt("cons")
            nc.scalar.activation(cons[0:84, :], bank[0:84, :], AF.Copy)
            nc.sync.dma_start(OUT[:, t, :, :], cons[0:84, :])

        load_ji(0)
        fin_prev = None
        for t in range(NT):
            prev = None
            for l in range(2, 9):
                if l == 3 and fin_prev is not None:
                    # previous tile's output consolidation, overlapped here
                    emit_compact(fin_prev, t - 1)
                    fin_prev = None
                emit_round(l, ROUND1, prev, ji)
                v, sq, c = emit_act(l)
                emit_round(l, ROUND2, prev, ji)
                if l == 2 and t + 1 < NT:
                    load_ji(t + 1)
                prev = emit_dve(l, v, sq, c, last=(l == 8))
            fin_prev = prev
        emit_compact(fin_prev, NT - 1)

    nc.finalize()
    return nc


_NC = None


def _get_nc():
    global _NC
    if _NC is None:
        _NC = _build_program()
    return _NC


def _host_pack(inputs):
    X32 = np.asarray(inputs["X"], dtype=np.float32)
    X = X32.astype(np.float64)
    Ws = [np.asarray(inputs[f"W{i}"], dtype=np.float64) for i in range(1, 9)]
    bs = [np.asarray(inputs[f"b{i}"], dtype=np.float64) for i in range(1, 9)]

    lb = float(X32[:, 0].min())
    ub = float(X32[:, 0].max())
    s = 2.0 / (ub - lb)
    cshift = -2.0 * lb / (ub - lb) - 1.0
    W1e = s * Ws[0]                      # [3,3]
    b1e = bs[0] + cshift * Ws[0].sum(axis=0)

    Z1 = X @ W1e + b1e                   # [N,3]
    y = np.tanh(Z1)
    sq = y * y
    f1 = 1.0 - sq
    f2 = -2.0 * y * f1
    f3 = f1 * (6.0 * sq - 2.0)
    ux, uy, ut = W1e[0], W1e[1], W1e[2]  # each [3]

    jets = np.empty((13, N_PTS, 3), dtype=np.float64)
    jets[0] = y
    jets[1] = f1 * ux
    jets[2] = f1 * uy
    jets[3] = f1 * ut
    jets[4] = f2 * (ux * ux)
    jets[5] = f2 * (ux * uy)
    jets[6] = f2 * (uy * uy)
    jets[7] = f2 * (ux * ut)
    jets[8] = f2 * (uy * ut)
    jets[9] = f3 * (ux * ux * ux)
    jets[10] = f3 * (ux * ux * uy)
    jets[11] = f3 * (ux * uy * uy)
    jets[12] = f3 * (uy * uy * uy)
    jets = jets.astype(np.float32)

    WBD = np.zeros((128, 7 * 128), dtype=np.float32)
    BBp = np.zeros((128, 7), dtype=np.float32)
    for l in range(2, 9):
        W = Ws[l - 1].astype(np.float32)
        b = bs[l - 1].astype(np.float32)
        din, dout = W.shape
        off = 128 * (l - 2)
        for g in range(GROUPS):
            WBD[GS * g:GS * g + din, off + GS * g:off + GS * g + dout] = W
            BBp[GS * g:GS * g + dout, l - 2] = b

    SELp = np.zeros((128, 12 * 84), dtype=np.float32)
    for i, nm in enumerate(TILE_ORDER):
        for r, (rnm, unit) in enumerate(ROWS):
            if rnm != nm:
                continue
            for g in range(GROUPS):
                SELp[GS * g + unit, 84 * i + 6 * r + g] = 1.0

    in_maps = []
    for k in range(N_CORES):
        A = jets[:, PTS_PER_CORE * k:PTS_PER_CORE * (k + 1), :]      # [13,8192,3]
        Ap = np.zeros((13, PAD_PTS, 3), dtype=np.float32)
        Ap[:, :PTS_PER_CORE] = A
        B = Ap.reshape(13, NT, GROUPS, F, 3)
        J1k = np.ascontiguousarray(
            B.transpose(2, 4, 0, 1, 3).reshape(GROUPS, 3, 13, NT * F))
        in_maps.append({"J1": J1k, "WBD": WBD, "BB": BBp, "SEL": SELp})
    return in_maps


_RUN = None
_RUN_BROKEN = False


def _get_runner():
    # cache the jitted executable so repeat kernel() calls skip re-trace,
    # re-lowering and NEFF recompilation (which dominate wall time)
    global _RUN
    if _RUN is not None:
        return _RUN
    import jax
    from concourse import bass2jax, mybir

    nc = _get_nc()
    bass2jax.install_neuronx_cc_hook()
    partition_name = nc.partition_id_tensor.name if nc.partition_id_tensor else None
    in_names, out_names, out_avals, zero_shapes = [], [], [], []
    for alloc in nc.m.functions[0].allocations:
        if not isinstance(alloc, mybir.MemoryLocationSet):
            continue
        name = alloc.memorylocations[0].name
        if alloc.kind == "ExternalInput":
            if name != partition_name:
                in_names.append(name)
        elif alloc.kind == "ExternalOutput":
            shape = tuple(alloc.tensor_shape)
            dtype = mybir.dt.np(alloc.dtype)
            out_names.append(name)
            out_avals.append(jax.core.ShapedArray(shape, dtype))
            zero_shapes.append((shape, dtype))
    n_params = len(in_names)
    n_outs = len(out_names)
    all_in = list(in_names) + list(out_names)
    if partition_name is not None:
        all_in.append(partition_name)
    donate = tuple(range(n_params, n_params + n_outs))

    def _body(*args):
        operands = list(args)
        if partition_name is not None:
            operands.append(bass2jax.partition_id_tensor())
        return tuple(bass2jax._bass_exec_p.bind(
            *operands,
            out_avals=tuple(out_avals),
            in_names=tuple(all_in),
            out_names=tuple(out_names),
            lowering_input_output_aliases=(),
            sim_require_finite=True,
            sim_require_nnan=True,
            nc=nc,
        ))

    devices = jax.devices()[:N_CORES]
    mesh = bass2jax.Mesh(np.asarray(devices), ("core",))
    in_specs = (bass2jax.PartitionSpec("core"),) * (n_params + n_outs)
    out_specs = (bass2jax.PartitionSpec("core"),) * n_outs
    sharded = jax.jit(
        bass2jax.shard_map(_body, mesh=mesh, in_specs=in_specs,
                           out_specs=out_specs, check_rep=False),
        donate_argnums=donate, keep_unused=True)
    _RUN = (sharded, in_names, out_avals, zero_shapes, n_params)
    return _RUN


def _run_cached(in_maps):
    sharded, in_names, out_avals, zero_shapes, n_params = _get_runner()
    concat_in = [
        np.concatenate([np.asarray(in_maps[c][nm]) for c in range(N_CORES)], axis=0)
        for nm in in_names]
    concat_zeros = [
        np.zeros((N_CORES * shape[0], *shape[1:]), dtype)
        for (shape, dtype) in zero_shapes]
    out_arrs = sharded(*concat_in, *concat_zeros)
    per = np.asarray(out_arrs[0]).reshape(N_CORES, *out_avals[0].shape)
    return per


def kernel(**inputs):
    global LAST_EXEC_NS, _RUN_BROKEN
    nc = _get_nc()
    in_maps = _host_pack(inputs)
    trace = bool(os.environ.get("BASS_KERNEL_TRACE"))
    if trace and importlib.util.find_spec("antenv.axon_hooks") is None:
        trace = False
    per = None
    if not trace and not _RUN_BROKEN:
        try:
            per = _run_cached(in_maps)
        except Exception:
            _RUN_BROKEN = True
            per = None
    if per is not None:
        LAST_EXEC_NS = None
        O = np.concatenate(
            [per[k].reshape(14, PAD_PTS)[:, :PTS_PER_CORE] for k in range(N_CORES)],
            axis=1).astype(np.float32)  # [14, 65536]
    else:
        from concourse.bass_utils import run_bass_kernel_spmd
        kw = {}
        if trace:
            kw["trace"] = True
            td = os.environ.get("BASS_KERNEL_TRACE_DIR")
            if td:
                kw["tmpdir"] = td
        res = run_bass_kernel_spmd(nc, in_maps, list(range(N_CORES)), **kw)
        LAST_EXEC_NS = res.exec_time_ns
        O = np.concatenate(
            [np.asarray(res.results[k]["OUT"]).reshape(14, PAD_PTS)[:, :PTS_PER_CORE]
             for k in range(N_CORES)],
            axis=1).astype(np.float32)  # [14, 65536]

    lam1 = np.float32(np.asarray(inputs["lam1"]).reshape(-1)[0])
    lam2 = np.float32(np.asarray(inputs["lam2"]).reshape(-1)[0])
    u = O[1].copy()
    vv = (-O[0]).astype(np.float32)
    p = O[11].copy()
    f_u = O[6] + lam1 * (O[1] * O[3] - O[0] * O[4]) + O[12] - lam2 * (O[8] + O[10])
    f_v = -O[5] + lam1 * (O[0] * O[3] - O[1] * O[2]) + O[13] + lam2 * (O[7] + O[9])
    return (u, vv, p[:, None].copy(),
            f_u.astype(np.float32), f_v.astype(np.float32))


# revision 42
# speedup vs baseline: 1.8003x; 1.0316x over previous
import importlib.util
import os
import sys

sys.path.insert(0, "/opt/trn_rl_repo")

import numpy as np
from contextlib import ExitStack

N_CORES = 8
N_PTS = 65536
PTS_PER_CORE = N_PTS // N_CORES  # 8192
GROUPS = 6                        # unit-groups of 20 partitions (120/128 used)
GS = 20                           # partition stride per group
F = 512                           # max points per instruction (PSUM bank cap)
NT = 3                            # super-tiles (ragged last: 512+512+342)
FPS = (512, 512, 342)
JOFF = (0, 512, 1024)
PTS_PER_GROUP = sum(FPS)          # 1366
PAD_PTS = GROUPS * PTS_PER_GROUP  # 8196 (only 4 pad points)
CH_LIST = ["val", "zx", "zy", "zt", "zxx", "zxy", "zyy", "zxt", "zyt",
           "zxxx", "zxxy", "zxyy", "zyyy"]
CH_IDX = {c: i for i, c in enumerate(CH_LIST)}
BANK = {"val": "A", "zx": "B", "zy": "C", "zt": "D", "zxx": "E", "zxy": "F",
        "zyy": "G", "zxt": "H", "zyt": "A", "zxxx": "B", "zxxy": "C",
        "zxyy": "D", "zyyy": "E"}
ROUND1 = ["val", "zx", "zy", "zt", "zxx", "zxy", "zyy", "zxt"]
ROUND2 = ["zyt", "zxxx", "zxxy", "zxyy", "zyyy"]
PIECES_OF = {
    "val": ["v"], "zx": ["hx"], "zy": ["hy"], "zt": ["ht"],
    "zxx": ["mxx", "nxx"], "zxy": ["mxy", "nxy"], "zyy": ["myy", "nyy"],
    "zxt": ["mnxt"], "zyt": ["mnyt"],
    "zxxx": ["r1xxx", "r2xxx", "r3xxx"], "zxxy": ["r1xxy", "r2xxy", "r3xxy"],
    "zxyy": ["r1xyy", "r2xyy", "r3xyy"], "zyyy": ["r1yyy", "r2yyy", "r3yyy"],
}
# output row -> (final-layer piece tile, unit offset within group)
ROWS = [("hx", 0), ("hy", 0), ("mxx", 0), ("mxy", 0), ("myy", 0),
        ("mnxt", 0), ("mnyt", 0), ("r1xxx", 0), ("r1xxy", 0), ("r1xyy", 0),
        ("r1yyy", 0), ("v", 1), ("hx", 1), ("hy", 1)]
TILE_ORDER = ["hx", "hy", "mxx", "mxy", "myy", "mnxt", "mnyt",
              "r1xxx", "r1xxy", "r1xyy", "r1yyy", "v"]

LAST_EXEC_NS = None


def _build_program():
    import concourse.bass as bass
    import concourse.bacc as bacc
    import concourse.tile as tile
    import concourse.mybir as mybir

    f32 = mybir.dt.float32
    AF = mybir.ActivationFunctionType
    ALU = mybir.AluOpType

    nc = bacc.Bacc("TRN2", target_bir_lowering=False, num_devices=N_CORES)
    J1 = nc.declare_dram_parameter("J1", [GROUPS, 3, 13, PTS_PER_GROUP], f32, isOutput=False)
    WBD = nc.declare_dram_parameter("WBD", [128, 7 * 128], f32, isOutput=False)
    BB = nc.declare_dram_parameter("BB", [128, 7], f32, isOutput=False)
    SEL = nc.declare_dram_parameter("SEL", [128, 12 * 84], f32, isOutput=False)
    OUT = nc.declare_dram_parameter("OUT", [14, GROUPS, PTS_PER_GROUP], f32, isOutput=True)

    with ExitStack() as ctx:
        tc = ctx.enter_context(tile.TileContext(nc))
        const = ctx.enter_context(tc.tile_pool(name="const", bufs=1))
        jets = ctx.enter_context(tc.tile_pool(name="jets", bufs=1))
        pieces = ctx.enter_context(tc.tile_pool(name="pieces", bufs=2))
        work = ctx.enter_context(tc.tile_pool(name="work", bufs=1))
        psum = ctx.enter_context(tc.tile_pool(name="psum", bufs=1, space=bass.MemorySpace.PSUM))

        wbd = const.tile([128, 7 * 128], f32, name="wbd")
        bb = const.tile([128, 7], f32, name="bb")
        sel = const.tile([128, 12 * 84], f32, name="sel")
        # layer-2 weights + bias first so compute can start as soon as the
        # first super-tile's jets land; remaining constants load behind them
        nc.sync.dma_start(wbd[:, 0:128], WBD[:, 0:128])
        nc.sync.dma_start(bb[:], BB[:])

        ps = {k: psum.tile([128, F], f32, name=f"ps{k}") for k in "ABCDEFGH"}

        def pt(name):
            return pieces.tile([128, F], f32, name=name)

        def wt(name):
            return work.tile([128, F], f32, name=name)

        def emit_round(l, chs, prev, ji, fp):
            off = 128 * (l - 2)
            lhsT = wbd[:, off:off + 128]
            for chn in chs:
                bank = ps[BANK[chn]]
                if l == 2:
                    srcs = [ji[:, CH_IDX[chn] * fp:(CH_IDX[chn] + 1) * fp]]
                else:
                    srcs = [prev[p][:, :fp] for p in PIECES_OF[chn]]
                for i, src in enumerate(srcs):
                    nc.tensor.matmul(bank[:, :fp], lhsT, src,
                                     start=(i == 0), stop=(i == len(srcs) - 1))

        def emit_act(l, fp):
            v = pt("v")
            nc.scalar.activation(v[:, :fp], ps["A"][:, :fp], AF.Tanh, bias=bb[:, l - 2:l - 1])
            sq = wt("sq")
            nc.scalar.activation(sq[:, :fp], v[:, :fp], AF.Square)
            c = {}
            for nm, bk in [("x", "B"), ("y", "C"), ("t", "D"),
                           ("xx", "E"), ("xy", "F"), ("yy", "G")]:
                cc = wt("c" + nm)
                nc.scalar.activation(cc[:, :fp], ps[bk][:, :fp], AF.Copy)
                c[nm] = cc
            return v, sq, c

        def emit_dve(l, v, sq, c, last, fp):
            def S(x):
                return x[:, :fp]
            P = {}
            f1 = wt("f1")
            nc.vector.tensor_scalar(S(f1), S(sq), -1.0, 1.0, ALU.mult, ALU.add)
            # free PSUM banks as early as possible
            nxt = wt("nxt"); nc.vector.tensor_mul(S(nxt), S(f1), S(ps["H"]))
            nyt = wt("nyt"); nc.vector.tensor_mul(S(nyt), S(f1), S(ps["A"]))
            r3 = {}
            for abc, bk in [("xxx", "B"), ("xxy", "C"), ("xyy", "D"), ("yyy", "E")]:
                r = pt("r3" + abc); nc.vector.tensor_mul(S(r), S(f1), S(ps[bk]))
                r3[abc] = r
            f2h = wt("f2h")
            nc.vector.scalar_tensor_tensor(S(f2h), S(sq), 1.0, S(v), ALU.subtract, ALU.mult)
            f3g = wt("f3g")
            nc.vector.scalar_tensor_tensor(S(f3g), S(sq), 1.0 / 3.0, S(f1), ALU.subtract, ALU.mult)
            hx = pt("hx"); nc.vector.tensor_mul(S(hx), S(f1), S(c["x"]))
            hy = pt("hy"); nc.vector.tensor_mul(S(hy), S(f1), S(c["y"]))
            ht = None
            if not last:
                ht = pt("ht"); nc.vector.tensor_mul(S(ht), S(f1), S(c["t"]))
            for ab, (a, b) in [("xx", ("x", "x")), ("xy", ("x", "y")), ("yy", ("y", "y")),
                               ("xt", ("x", "t")), ("yt", ("y", "t"))]:
                pp = wt("p" + ab); nc.gpsimd.tensor_mul(S(pp), S(c[a]), S(c[b]))
                P[ab] = pp
            m = {}
            for ab in ["xx", "xy", "yy", "xt", "yt"]:
                if ab in ("xx", "xy", "yy"):
                    mm = pt("m" + ab)
                elif last:
                    # must outlive this tile (read by deferred compaction)
                    mm = pt("mn" + ab)
                else:
                    mm = wt("m" + ab)
                nc.vector.scalar_tensor_tensor(S(mm), S(P[ab]), 2.0, S(f2h), ALU.mult, ALU.mult)
                m[ab] = mm
            n = {"xt": nxt, "yt": nyt}
            for ab, eng in [("xx", nc.gpsimd), ("xy", nc.gpsimd), ("yy", nc.vector)]:
                nn = pt("n" + ab); eng.tensor_mul(S(nn), S(f1), S(c[ab]))
                n[ab] = nn
            q = {}
            for qi, (a, b) in [("1", ("xx", "x")), ("2", ("xx", "y")), ("3", ("xy", "x")),
                               ("4", ("xy", "y")), ("5", ("yy", "x")), ("6", ("yy", "y"))]:
                qq = wt("q" + qi); nc.gpsimd.tensor_mul(S(qq), S(c[a]), S(c[b]))
                q[qi] = qq
            sxxy = wt("sxxy")
            nc.vector.scalar_tensor_tensor(S(sxxy), S(q["3"]), 2.0, S(q["2"]), ALU.mult, ALU.add)
            sxyy = wt("sxyy")
            nc.vector.scalar_tensor_tensor(S(sxyy), S(q["4"]), 2.0, S(q["5"]), ALU.mult, ALU.add)
            T = {}
            for abc, (pab, a) in [("xxx", ("xx", "x")), ("xxy", ("xx", "y")),
                                  ("xyy", ("yy", "x")), ("yyy", ("yy", "y"))]:
                tt = wt("t" + abc); nc.gpsimd.tensor_mul(S(tt), S(P[pab]), S(c[a]))
                T[abc] = tt
            r1 = {}
            for abc in ["xxx", "xxy", "xyy", "yyy"]:
                rr = pt("r1" + abc)
                nc.vector.scalar_tensor_tensor(S(rr), S(T[abc]), 6.0, S(f3g), ALU.mult, ALU.mult)
                r1[abc] = rr
            r2 = {}
            for abc, (src, k) in [("xxx", (q["1"], 6.0)), ("xxy", (sxxy, 2.0)),
                                  ("xyy", (sxyy, 2.0)), ("yyy", (q["6"], 6.0))]:
                rr = pt("r2" + abc)
                nc.vector.scalar_tensor_tensor(S(rr), S(src), k, S(f2h), ALU.mult, ALU.mult)
                r2[abc] = rr

            if not last:
                out = {"v": v, "hx": hx, "hy": hy, "ht": ht}
                for ab in ["xt", "yt"]:
                    z = pt("mn" + ab)
                    nc.vector.tensor_add(S(z), S(m[ab]), S(n[ab]))
                    out["mn" + ab] = z
                for ab in ["xx", "xy", "yy"]:
                    out["m" + ab] = m[ab]
                    out["n" + ab] = n[ab]
                for abc in ["xxx", "xxy", "xyy", "yyy"]:
                    out["r1" + abc] = r1[abc]
                    out["r2" + abc] = r2[abc]
                    out["r3" + abc] = r3[abc]
                return out
            # last layer: fold pieces into final jets (in-place adds)
            for ab in ["xx", "xy", "yy", "xt", "yt"]:
                nc.vector.tensor_add(S(m[ab]), S(m[ab]), S(n[ab]))
            for abc in ["xxx", "xxy", "xyy", "yyy"]:
                nc.vector.tensor_add(S(r1[abc]), S(r1[abc]), S(r2[abc]))
                nc.vector.tensor_add(S(r1[abc]), S(r1[abc]), S(r3[abc]))
            return {"hx": hx, "hy": hy, "mxx": m["xx"], "mxy": m["xy"],
                    "myy": m["yy"], "mnxt": m["xt"], "mnyt": m["yt"],
                    "r1xxx": r1["xxx"], "r1xxy": r1["xxy"], "r1xyy": r1["xyy"],
                    "r1yyy": r1["yyy"], "v": v}

        ji = jets.tile([128, 13 * F], f32, name="ji")
        nc.vector.memset(ji[:], 0.0)

        def load_ji(t):
            # per-tile packed layout: channel ch occupies ji cols [ch*fp, (ch+1)*fp)
            fp = FPS[t]
            for g in range(GROUPS):
                nc.sync.dma_start(ji[GS * g:GS * g + 3, 0:13 * fp],
                                  J1[g, :, :, JOFF[t]:JOFF[t] + fp])

        def emit_compact(fin, t, fp):
            # gather the 14 output rows x 6 groups into partitions 6*row+g of
            # one PSUM bank via selection matmuls, then 1 copy + 1 DMA descriptor
            bank = ps["H"]
            for i, nm in enumerate(TILE_ORDER):
                nc.tensor.matmul(bank[0:84, :fp], sel[:, 84 * i:84 * (i + 1)],
                                 fin[nm][:, :fp], start=(i == 0), stop=(i == 11))
            cons = wt("cons")
            nc.scalar.activation(cons[0:84, :fp], bank[0:84, :fp], AF.Copy)
            nc.sync.dma_start(OUT[:, :, JOFF[t]:JOFF[t] + fp], cons[0:84, :fp])

        load_ji(0)
        # non-critical constants load behind the first jets
        nc.sync.dma_start(wbd[:, 128:7 * 128], WBD[:, 128:7 * 128])
        nc.sync.dma_start(sel[:], SEL[:])
        fin_prev = None
        for t in range(NT):
            fp = FPS[t]
            prev = None
            for l in range(2, 9):
                if l == 3 and fin_prev is not None:
                    # previous tile's output consolidation, overlapped here
                    emit_compact(*fin_prev)
                    fin_prev = None
                emit_round(l, ROUND1, prev, ji, fp)
                v, sq, c = emit_act(l, fp)
                emit_round(l, ROUND2, prev, ji, fp)
                if l == 2 and t + 1 < NT:
                    load_ji(t + 1)
                prev = emit_dve(l, v, sq, c, last=(l == 8), fp=fp)
            fin_prev = (prev, t, fp)
        emit_compact(*fin_prev)

    nc.finalize()
    return nc


_NC = None


def _get_nc():
    global _NC
    if _NC is None:
        _NC = _build_program()
    return _NC


def _host_pack(inputs):
    X32 = np.asarray(inputs["X"], dtype=np.float32)
    X = X32.astype(np.float64)
    Ws = [np.asarray(inputs[f"W{i}"], dtype=np.float64) for i in range(1, 9)]
    bs = [np.asarray(inputs[f"b{i}"], dtype=np.float64) for i in range(1, 9)]

    lb = float(X32[:, 0].min())
    ub = float(X32[:, 0].max())
    s = 2.0 / (ub - lb)
    cshift = -2.0 * lb / (ub - lb) - 1.0
    W1e = s * Ws[0]                      # [3,3]
    b1e = bs[0] + cshift * Ws[0].sum(axis=0)

    Z1 = X @ W1e + b1e                   # [N,3]
    y = np.tanh(Z1)
    sq = y * y
    f1 = 1.0 - sq
    f2 = -2.0 * y * f1
    f3 = f1 * (6.0 * sq - 2.0)
    ux, uy, ut = W1e[0], W1e[1], W1e[2]  # each [3]

    jets = np.empty((13, N_PTS, 3), dtype=np.float64)
    jets[0] = y
    jets[1] = f1 * ux
    jets[2] = f1 * uy
    jets[3] = f1 * ut
    jets[4] = f2 * (ux * ux)
    jets[5] = f2 * (ux * uy)
    jets[6] = f2 * (uy * uy)
    jets[7] = f2 * (ux * ut)
    jets[8] = f2 * (uy * ut)
    jets[9] = f3 * (ux * ux * ux)
    jets[10] = f3 * (ux * ux * uy)
    jets[11] = f3 * (ux * uy * uy)
    jets[12] = f3 * (uy * uy * uy)
    jets = jets.astype(np.float32)

    WBD = np.zeros((128, 7 * 128), dtype=np.float32)
    BBp = np.zeros((128, 7), dtype=np.float32)
    for l in range(2, 9):
        W = Ws[l - 1].astype(np.float32)
        b = bs[l - 1].astype(np.float32)
        din, dout = W.shape
        off = 128 * (l - 2)
        for g in range(GROUPS):
            WBD[GS * g:GS * g + din, off + GS * g:off + GS * g + dout] = W
            BBp[GS * g:GS * g + dout, l - 2] = b

    SELp = np.zeros((128, 12 * 84), dtype=np.float32)
    for i, nm in enumerate(TILE_ORDER):
        for r, (rnm, unit) in enumerate(ROWS):
            if rnm != nm:
                continue
            for g in range(GROUPS):
                SELp[GS * g + unit, 84 * i + 6 * r + g] = 1.0

    in_maps = []
    for k in range(N_CORES):
        A = jets[:, PTS_PER_CORE * k:PTS_PER_CORE * (k + 1), :]      # [13,8192,3]
        Ap = np.zeros((13, PAD_PTS, 3), dtype=np.float32)
        Ap[:, :PTS_PER_CORE] = A
        B = Ap.reshape(13, GROUPS, PTS_PER_GROUP, 3)
        J1k = np.ascontiguousarray(B.transpose(1, 3, 0, 2))  # [G,3,13,1366]
        in_maps.append({"J1": J1k, "WBD": WBD, "BB": BBp, "SEL": SELp})
    return in_maps


_RUN = None
_RUN_BROKEN = False


def _get_runner():
    # cache the jitted executable so repeat kernel() calls skip re-trace,
    # re-lowering and NEFF recompilation (which dominate wall time)
    global _RUN
    if _RUN is not None:
        return _RUN
    import jax
    from concourse import bass2jax, mybir

    nc = _get_nc()
    bass2jax.install_neuronx_cc_hook()
    partition_name = nc.partition_id_tensor.name if nc.partition_id_tensor else None
    in_names, out_names, out_avals, zero_shapes = [], [], [], []
    for alloc in nc.m.functions[0].allocations:
        if not isinstance(alloc, mybir.MemoryLocationSet):
            continue
        name = alloc.memorylocations[0].name
        if alloc.kind == "ExternalInput":
            if name != partition_name:
                in_names.append(name)
        elif alloc.kind == "ExternalOutput":
            shape = tuple(alloc.tensor_shape)
            dtype = mybir.dt.np(alloc.dtype)
            out_names.append(name)
            out_avals.append(jax.core.ShapedArray(shape, dtype))
            zero_shapes.append((shape, dtype))
    n_params = len(in_names)
    n_outs = len(out_names)
    all_in = list(in_names) + list(out_names)
    if partition_name is not None:
        all_in.append(partition_name)
    donate = tuple(range(n_params, n_params + n_outs))

    def _body(*args):
        operands = list(args)
        if partition_name is not None:
            operands.append(bass2jax.partition_id_tensor())
        return tuple(bass2jax._bass_exec_p.bind(
            *operands,
            out_avals=tuple(out_avals),
            in_names=tuple(all_in),
            out_names=tuple(out_names),
            lowering_input_output_aliases=(),
            sim_require_finite=True,
            sim_require_nnan=True,
            nc=nc,
        ))

    devices = jax.devices()[:N_CORES]
    mesh = bass2jax.Mesh(np.asarray(devices), ("core",))
    in_specs = (bass2jax.PartitionSpec("core"),) * (n_params + n_outs)
    out_specs = (bass2jax.PartitionSpec("core"),) * n_outs
    sharded = jax.jit(
        bass2jax.shard_map(_body, mesh=mesh, in_specs=in_specs,
                           out_specs=out_specs, check_rep=False),
        donate_argnums=donate, keep_unused=True)
    _RUN = (sharded, in_names, out_avals, zero_shapes, n_params)
    return _RUN


def _run_cached(in_maps):
    sharded, in_names, out_avals, zero_shapes, n_params = _get_runner()
    concat_in = [
        np.concatenate([np.asarray(in_maps[c][nm]) for c in range(N_CORES)], axis=0)
        for nm in in_names]
    concat_zeros = [
        np.zeros((N_CORES * shape[0], *shape[1:]), dtype)
        for (shape, dtype) in zero_shapes]
    out_arrs = sharded(*concat_in, *concat_zeros)
    per = np.asarray(out_arrs[0]).reshape(N_CORES, *out_avals[0].shape)
    return per


def kernel(**inputs):
    global LAST_EXEC_NS, _RUN_BROKEN
    nc = _get_nc()
    in_maps = _host_pack(inputs)
    trace = bool(os.environ.get("BASS_KERNEL_TRACE"))
    if trace and importlib.util.find_spec("antenv.axon_hooks") is None:
        trace = False
    per = None
    if not trace and not _RUN_BROKEN:
        try:
            per = _run_cached(in_maps)
        except Exception:
            _RUN_BROKEN = True
            per = None
    if per is not None:
        LAST_EXEC_NS = None
        O = np.concatenate(
            [per[k].reshape(14, PAD_PTS)[:, :PTS_PER_CORE] for k in range(N_CORES)],
            axis=1).astype(np.float32)  # [14, 65536]
    else:
        from concourse.bass_utils import run_bass_kernel_spmd
        kw = {}
        if trace:
            kw["trace"] = True
            td = os.environ.get("BASS_KERNEL_TRACE_DIR")
            if td:
                kw["tmpdir"] = td
        res = run_bass_kernel_spmd(nc, in_maps, list(range(N_CORES)), **kw)
        LAST_EXEC_NS = res.exec_time_ns
        O = np.concatenate(
            [np.asarray(res.results[k]["OUT"]).reshape(14, PAD_PTS)[:, :PTS_PER_CORE]
             for k in range(N_CORES)],
            axis=1).astype(np.float32)  # [14, 65536]

    lam1 = np.float32(np.asarray(inputs["lam1"]).reshape(-1)[0])
    lam2 = np.float32(np.asarray(inputs["lam2"]).reshape(-1)[0])
    u = O[1].copy()
    vv = (-O[0]).astype(np.float32)
    p = O[11].copy()
    f_u = O[6] + lam1 * (O[1] * O[3] - O[0] * O[4]) + O[12] - lam2 * (O[8] + O[10])
    f_v = -O[5] + lam1 * (O[0] * O[3] - O[1] * O[2]) + O[13] + lam2 * (O[7] + O[9])
    return (u, vv, p[:, None].copy(),
            f_u.astype(np.float32), f_v.astype(np.float32))


# revision 51
# speedup vs baseline: 1.8092x; 1.0049x over previous
import importlib.util
import os
import sys

sys.path.insert(0, "/opt/trn_rl_repo")

import numpy as np
from contextlib import ExitStack

N_CORES = 8
N_PTS = 65536
PTS_PER_CORE = N_PTS // N_CORES  # 8192
GROUPS = 6                        # unit-groups of 20 partitions (120/128 used)
GS = 20                           # partition stride per group
F = 512                           # max points per instruction (PSUM bank cap)
NT = 3                            # super-tiles (ragged last: 512+512+342)
FPS = (512, 512, 342)
JOFF = (0, 512, 1024)
PTS_PER_GROUP = sum(FPS)          # 1366
PAD_PTS = GROUPS * PTS_PER_GROUP  # 8196 (only 4 pad points)
CH_LIST = ["val", "zx", "zy", "zt", "zxx", "zxy", "zyy", "zxt", "zyt",
           "zxxx", "zxxy", "zxyy", "zyyy"]
CH_IDX = {c: i for i, c in enumerate(CH_LIST)}
BANK = {"val": "A", "zx": "B", "zy": "C", "zt": "D", "zxx": "E", "zxy": "F",
        "zyy": "G", "zxt": "H", "zyt": "A", "zxxx": "B", "zxxy": "C",
        "zxyy": "D", "zyyy": "E"}
ROUND1 = ["val", "zx", "zy", "zt", "zxx", "zxy", "zyy", "zxt"]
ROUND2 = ["zyt", "zxxx", "zxxy", "zxyy", "zyyy"]
PIECES_OF = {
    "val": ["v"], "zx": ["hx"], "zy": ["hy"], "zt": ["ht"],
    "zxx": ["mxx", "nxx"], "zxy": ["mxy", "nxy"], "zyy": ["myy", "nyy"],
    "zxt": ["mnxt"], "zyt": ["mnyt"],
    "zxxx": ["r1xxx", "r2xxx", "r3xxx"], "zxxy": ["r1xxy", "r2xxy", "r3xxy"],
    "zxyy": ["r1xyy", "r2xyy", "r3xyy"], "zyyy": ["r1yyy", "r2yyy", "r3yyy"],
}
# output row -> (final-layer piece tile, unit offset within group)
ROWS = [("hx", 0), ("hy", 0), ("mxx", 0), ("mxy", 0), ("myy", 0),
        ("mnxt", 0), ("mnyt", 0), ("r1xxx", 0), ("r1xxy", 0), ("r1xyy", 0),
        ("r1yyy", 0), ("v", 1), ("hx", 1), ("hy", 1)]
TILE_ORDER = ["hx", "hy", "mxx", "mxy", "myy", "mnxt", "mnyt",
              "r1xxx", "r1xxy", "r1xyy", "r1yyy", "v"]

LAST_EXEC_NS = None


def _build_program():
    import concourse.bass as bass
    import concourse.bacc as bacc
    import concourse.tile as tile
    import concourse.mybir as mybir

    f32 = mybir.dt.float32
    AF = mybir.ActivationFunctionType
    ALU = mybir.AluOpType

    nc = bacc.Bacc("TRN2", target_bir_lowering=False, num_devices=N_CORES)
    J1 = nc.declare_dram_parameter("J1", [GROUPS, 3, 13, PTS_PER_GROUP], f32, isOutput=False)
    WBD = nc.declare_dram_parameter("WBD", [128, 7 * 128], f32, isOutput=False)
    BB = nc.declare_dram_parameter("BB", [128, 7], f32, isOutput=False)
    SEL = nc.declare_dram_parameter("SEL", [128, 12 * 84], f32, isOutput=False)
    OUT = nc.declare_dram_parameter("OUT", [14, GROUPS, PTS_PER_GROUP], f32, isOutput=True)

    with ExitStack() as ctx:
        tc = ctx.enter_context(tile.TileContext(nc))
        const = ctx.enter_context(tc.tile_pool(name="const", bufs=1))
        jets = ctx.enter_context(tc.tile_pool(name="jets", bufs=1))
        pieces = ctx.enter_context(tc.tile_pool(name="pieces", bufs=2))
        work = ctx.enter_context(tc.tile_pool(name="work", bufs=1))
        psum = ctx.enter_context(tc.tile_pool(name="psum", bufs=1, space=bass.MemorySpace.PSUM))

        wbd = const.tile([128, 7 * 128], f32, name="wbd")
        bb = const.tile([128, 7], f32, name="bb")
        sel = const.tile([128, 12 * 84], f32, name="sel")
        # layer-2 weights + bias first so compute can start as soon as the
        # first super-tile's jets land; remaining constants load behind them
        nc.sync.dma_start(wbd[:, 0:128], WBD[:, 0:128])
        nc.sync.dma_start(bb[:], BB[:])

        ps = {k: psum.tile([128, F], f32, name=f"ps{k}") for k in "ABCDEFGH"}

        def pt(name):
            return pieces.tile([128, F], f32, name=name)

        def wt(name):
            return work.tile([128, F], f32, name=name)

        def emit_round(l, chs, prev, ji, fp):
            off = 128 * (l - 2)
            lhsT = wbd[:, off:off + 128]
            for chn in chs:
                bank = ps[BANK[chn]]
                if l == 2:
                    srcs = [ji[:, CH_IDX[chn] * fp:(CH_IDX[chn] + 1) * fp]]
                else:
                    srcs = [prev[p][:, :fp] for p in PIECES_OF[chn]]
                for i, src in enumerate(srcs):
                    nc.tensor.matmul(bank[:, :fp], lhsT, src,
                                     start=(i == 0), stop=(i == len(srcs) - 1))

        def emit_act(l, fp):
            v = pt("v")
            nc.scalar.activation(v[:, :fp], ps["A"][:, :fp], AF.Tanh, bias=bb[:, l - 2:l - 1])
            sq = wt("sq")
            nc.scalar.activation(sq[:, :fp], v[:, :fp], AF.Square)
            f1 = pt("f1")
            nc.scalar.activation(f1[:, :fp], sq[:, :fp], AF.Copy, bias=1.0, scale=-1.0)
            c = {}
            for nm, bk in [("x", "B"), ("y", "C"), ("t", "D"),
                           ("xx", "E"), ("xy", "F"), ("yy", "G")]:
                cc = wt("c" + nm)
                nc.scalar.activation(cc[:, :fp], ps[bk][:, :fp], AF.Copy)
                c[nm] = cc
            return v, sq, f1, c

        def emit_dve(l, v, sq, f1, c, last, fp):
            def S(x):
                return x[:, :fp]
            P = {}
            # free PSUM banks as early as possible
            nxt = wt("nxt"); nc.vector.tensor_mul(S(nxt), S(f1), S(ps["H"]))
            nyt = wt("nyt"); nc.vector.tensor_mul(S(nyt), S(f1), S(ps["A"]))
            r3 = {}
            for abc, bk in [("xxx", "B"), ("xxy", "C"), ("xyy", "D"), ("yyy", "E")]:
                r = pt("r3" + abc); nc.vector.tensor_mul(S(r), S(f1), S(ps[bk]))
                r3[abc] = r
            f2h = wt("f2h")
            nc.vector.scalar_tensor_tensor(S(f2h), S(sq), 1.0, S(v), ALU.subtract, ALU.mult)
            f3g = wt("f3g")
            nc.vector.scalar_tensor_tensor(S(f3g), S(sq), 1.0 / 3.0, S(f1), ALU.subtract, ALU.mult)
            hx = pt("hx"); nc.vector.tensor_mul(S(hx), S(f1), S(c["x"]))
            hy = pt("hy"); nc.vector.tensor_mul(S(hy), S(f1), S(c["y"]))
            ht = None
            if not last:
                ht = pt("ht"); nc.vector.tensor_mul(S(ht), S(f1), S(c["t"]))
            for ab, (a, b) in [("xx", ("x", "x")), ("xy", ("x", "y")), ("yy", ("y", "y")),
                               ("xt", ("x", "t")), ("yt", ("y", "t"))]:
                pp = wt("p" + ab); nc.gpsimd.tensor_mul(S(pp), S(c[a]), S(c[b]))
                P[ab] = pp
            m = {}
            for ab in ["xx", "xy", "yy", "xt", "yt"]:
                if ab in ("xx", "xy", "yy"):
                    mm = pt("m" + ab)
                elif last:
                    # must outlive this tile (read by deferred compaction)
                    mm = pt("mn" + ab)
                else:
                    mm = wt("m" + ab)
                nc.vector.scalar_tensor_tensor(S(mm), S(P[ab]), 2.0, S(f2h), ALU.mult, ALU.mult)
                m[ab] = mm
            n = {"xt": nxt, "yt": nyt}
            for ab, eng in [("xx", nc.gpsimd), ("xy", nc.gpsimd), ("yy", nc.vector)]:
                nn = pt("n" + ab); eng.tensor_mul(S(nn), S(f1), S(c[ab]))
                n[ab] = nn
            q = {}
            for qi, (a, b) in [("1", ("xx", "x")), ("2", ("xx", "y")), ("3", ("xy", "x")),
                               ("4", ("xy", "y")), ("5", ("yy", "x")), ("6", ("yy", "y"))]:
                qq = wt("q" + qi); nc.gpsimd.tensor_mul(S(qq), S(c[a]), S(c[b]))
                q[qi] = qq
            sxxy = wt("sxxy")
            nc.vector.scalar_tensor_tensor(S(sxxy), S(q["3"]), 2.0, S(q["2"]), ALU.mult, ALU.add)
            sxyy = wt("sxyy")
            nc.vector.scalar_tensor_tensor(S(sxyy), S(q["4"]), 2.0, S(q["5"]), ALU.mult, ALU.add)
            T = {}
            for abc, (pab, a) in [("xxx", ("xx", "x")), ("xxy", ("xx", "y")),
                                  ("xyy", ("yy", "x")), ("yyy", ("yy", "y"))]:
                tt = wt("t" + abc); nc.gpsimd.tensor_mul(S(tt), S(P[pab]), S(c[a]))
                T[abc] = tt
            r1 = {}
            for abc in ["xxx", "xxy", "xyy", "yyy"]:
                rr = pt("r1" + abc)
                nc.vector.scalar_tensor_tensor(S(rr), S(T[abc]), 6.0, S(f3g), ALU.mult, ALU.mult)
                r1[abc] = rr
            r2 = {}
            for abc, (src, k) in [("xxx", (q["1"], 6.0)), ("xxy", (sxxy, 2.0)),
                                  ("xyy", (sxyy, 2.0)), ("yyy", (q["6"], 6.0))]:
                rr = pt("r2" + abc)
                nc.vector.scalar_tensor_tensor(S(rr), S(src), k, S(f2h), ALU.mult, ALU.mult)
                r2[abc] = rr

            if not last:
                out = {"v": v, "hx": hx, "hy": hy, "ht": ht}
                for ab in ["xt", "yt"]:
                    z = pt("mn" + ab)
                    nc.vector.tensor_add(S(z), S(m[ab]), S(n[ab]))
                    out["mn" + ab] = z
                for ab in ["xx", "xy", "yy"]:
                    out["m" + ab] = m[ab]
                    out["n" + ab] = n[ab]
                for abc in ["xxx", "xxy", "xyy", "yyy"]:
                    out["r1" + abc] = r1[abc]
                    out["r2" + abc] = r2[abc]
                    out["r3" + abc] = r3[abc]
                return out
            # last layer: fold pieces into final jets (in-place adds)
            for ab in ["xx", "xy", "yy", "xt", "yt"]:
                nc.vector.tensor_add(S(m[ab]), S(m[ab]), S(n[ab]))
            for abc in ["xxx", "xxy", "xyy", "yyy"]:
                nc.vector.tensor_add(S(r1[abc]), S(r1[abc]), S(r2[abc]))
                nc.vector.tensor_add(S(r1[abc]), S(r1[abc]), S(r3[abc]))
            return {"hx": hx, "hy": hy, "mxx": m["xx"], "mxy": m["xy"],
                    "myy": m["yy"], "mnxt": m["xt"], "mnyt": m["yt"],
                    "r1xxx": r1["xxx"], "r1xxy": r1["xxy"], "r1xyy": r1["xyy"],
                    "r1yyy": r1["yyy"], "v": v}

        ji = jets.tile([128, 13 * F], f32, name="ji")
        nc.vector.memset(ji[:], 0.0)

        def load_ji(t):
            # per-tile packed layout: channel ch occupies ji cols [ch*fp, (ch+1)*fp)
            fp = FPS[t]
            for g in range(GROUPS):
                nc.sync.dma_start(ji[GS * g:GS * g + 3, 0:13 * fp],
                                  J1[g, :, :, JOFF[t]:JOFF[t] + fp])

        def emit_compact(fin, t, fp):
            # gather the 14 output rows x 6 groups into partitions 6*row+g of
            # one PSUM bank via selection matmuls, then 1 copy + 1 DMA descriptor
            bank = ps["H"]
            for i, nm in enumerate(TILE_ORDER):
                nc.tensor.matmul(bank[0:84, :fp], sel[:, 84 * i:84 * (i + 1)],
                                 fin[nm][:, :fp], start=(i == 0), stop=(i == 11))
            cons = wt("cons")
            nc.scalar.activation(cons[0:84, :fp], bank[0:84, :fp], AF.Copy)
            nc.sync.dma_start(OUT[:, :, JOFF[t]:JOFF[t] + fp], cons[0:84, :fp])

        load_ji(0)
        # non-critical constants load behind the first jets
        nc.sync.dma_start(wbd[:, 128:7 * 128], WBD[:, 128:7 * 128])
        nc.sync.dma_start(sel[:], SEL[:])
        fin_prev = None
        for t in range(NT):
            fp = FPS[t]
            prev = None
            for l in range(2, 9):
                if l == 3 and fin_prev is not None:
                    # previous tile's output consolidation, overlapped here
                    emit_compact(*fin_prev)
                    fin_prev = None
                emit_round(l, ROUND1, prev, ji, fp)
                v, sq, f1, c = emit_act(l, fp)
                emit_round(l, ROUND2, prev, ji, fp)
                if l == 2 and t + 1 < NT:
                    load_ji(t + 1)
                prev = emit_dve(l, v, sq, f1, c, last=(l == 8), fp=fp)
            fin_prev = (prev, t, fp)
        emit_compact(*fin_prev)

    nc.finalize()
    return nc


_NC = None


def _get_nc():
    global _NC
    if _NC is None:
        _NC = _build_program()
    return _NC


def _host_pack(inputs):
    X32 = np.asarray(inputs["X"], dtype=np.float32)
    X = X32.astype(np.float64)
    Ws = [np.asarray(inputs[f"W{i}"], dtype=np.float64) for i in range(1, 9)]
    bs = [np.asarray(inputs[f"b{i}"], dtype=np.float64) for i in range(1, 9)]

    lb = float(X32[:, 0].min())
    ub = float(X32[:, 0].max())
    s = 2.0 / (ub - lb)
    cshift = -2.0 * lb / (ub - lb) - 1.0
    W1e = s * Ws[0]                      # [3,3]
    b1e = bs[0] + cshift * Ws[0].sum(axis=0)

    Z1 = X @ W1e + b1e                   # [N,3]
    y = np.tanh(Z1)
    sq = y * y
    f1 = 1.0 - sq
    f2 = -2.0 * y * f1
    f3 = f1 * (6.0 * sq - 2.0)
    ux, uy, ut = W1e[0], W1e[1], W1e[2]  # each [3]

    jets = np.empty((13, N_PTS, 3), dtype=np.float64)
    jets[0] = y
    jets[1] = f1 * ux
    jets[2] = f1 * uy
    jets[3] = f1 * ut
    jets[4] = f2 * (ux * ux)
    jets[5] = f2 * (ux * uy)
    jets[6] = f2 * (uy * uy)
    jets[7] = f2 * (ux * ut)
    jets[8] = f2 * (uy * ut)
    jets[9] = f3 * (ux * ux * ux)
    jets[10] = f3 * (ux * ux * uy)
    jets[11] = f3 * (ux * uy * uy)
    jets[12] = f3 * (uy * uy * uy)
    jets = jets.astype(np.float32)

    WBD = np.zeros((128, 7 * 128), dtype=np.float32)
    BBp = np.zeros((128, 7), dtype=np.float32)
    for l in range(2, 9):
        W = Ws[l - 1].astype(np.float32)
        b = bs[l - 1].astype(np.float32)
        din, dout = W.shape
        off = 128 * (l - 2)
        for g in range(GROUPS):
            WBD[GS * g:GS * g + din, off + GS * g:off + GS * g + dout] = W
            BBp[GS * g:GS * g + dout, l - 2] = b

    SELp = np.zeros((128, 12 * 84), dtype=np.float32)
    for i, nm in enumerate(TILE_ORDER):
        for r, (rnm, unit) in enumerate(ROWS):
            if rnm != nm:
                continue
            for g in range(GROUPS):
                SELp[GS * g + unit, 84 * i + 6 * r + g] = 1.0

    in_maps = []
    for k in range(N_CORES):
        A = jets[:, PTS_PER_CORE * k:PTS_PER_CORE * (k + 1), :]      # [13,8192,3]
        Ap = np.zeros((13, PAD_PTS, 3), dtype=np.float32)
        Ap[:, :PTS_PER_CORE] = A
        B = Ap.reshape(13, GROUPS, PTS_PER_GROUP, 3)
        J1k = np.ascontiguousarray(B.transpose(1, 3, 0, 2))  # [G,3,13,1366]
        in_maps.append({"J1": J1k, "WBD": WBD, "BB": BBp, "SEL": SELp})
    return in_maps


_RUN = None
_RUN_BROKEN = False


def _get_runner():
    # cache the jitted executable so repeat kernel() calls skip re-trace,
    # re-lowering and NEFF recompilation (which dominate wall time)
    global _RUN
    if _RUN is not None:
        return _RUN
    import jax
    from concourse import bass2jax, mybir

    nc = _get_nc()
    bass2jax.install_neuronx_cc_hook()
    partition_name = nc.partition_id_tensor.name if nc.partition_id_tensor else None
    in_names, out_names, out_avals, zero_shapes = [], [], [], []
    for alloc in nc.m.functions[0].allocations:
        if not isinstance(alloc, mybir.MemoryLocationSet):
            continue
        name = alloc.memorylocations[0].name
        if alloc.kind == "ExternalInput":
            if name != partition_name:
                in_names.append(name)
        elif alloc.kind == "ExternalOutput":
            shape = tuple(alloc.tensor_shape)
            dtype = mybir.dt.np(alloc.dtype)
            out_names.append(name)
            out_avals.append(jax.core.ShapedArray(shape, dtype))
            zero_shapes.append((shape, dtype))
    n_params = len(in_names)
    n_outs = len(out_names)
    all_in = list(in_names) + list(out_names)
    if partition_name is not None:
        all_in.append(partition_name)
    donate = tuple(range(n_params, n_params + n_outs))

    def _body(*args):
        operands = list(args)
        if partition_name is not None:
            operands.append(bass2jax.partition_id_tensor())
        return tuple(bass2jax._bass_exec_p.bind(
            *operands,
            out_avals=tuple(out_avals),
            in_names=tuple(all_in),
            out_names=tuple(out_names),
            lowering_input_output_aliases=(),
            sim_require_finite=True,
            sim_require_nnan=True,
            nc=nc,
        ))

    devices = jax.devices()[:N_CORES]
    mesh = bass2jax.Mesh(np.asarray(devices), ("core",))
    in_specs = (bass2jax.PartitionSpec("core"),) * (n_params + n_outs)
    out_specs = (bass2jax.PartitionSpec("core"),) * n_outs
    sharded = jax.jit(
        bass2jax.shard_map(_body, mesh=mesh, in_specs=in_specs,
                           out_specs=out_specs, check_rep=False),
        donate_argnums=donate, keep_unused=True)
    _RUN = (sharded, in_names, out_avals, zero_shapes, n_params)
    return _RUN


def _run_cached(in_maps):
    sharded, in_names, out_avals, zero_shapes, n_params = _get_runner()
    concat_in = [
        np.concatenate([np.asarray(in_maps[c][nm]) for c in range(N_CORES)], axis=0)
        for nm in in_names]
    concat_zeros = [
        np.zeros((N_CORES * shape[0], *shape[1:]), dtype)
        for (shape, dtype) in zero_shapes]
    out_arrs = sharded(*concat_in, *concat_zeros)
    per = np.asarray(out_arrs[0]).reshape(N_CORES, *out_avals[0].shape)
    return per


def kernel(**inputs):
    global LAST_EXEC_NS, _RUN_BROKEN
    nc = _get_nc()
    in_maps = _host_pack(inputs)
    trace = bool(os.environ.get("BASS_KERNEL_TRACE"))
    if trace and importlib.util.find_spec("antenv.axon_hooks") is None:
        trace = False
    per = None
    if not trace and not _RUN_BROKEN:
        try:
            per = _run_cached(in_maps)
        except Exception:
            _RUN_BROKEN = True
            per = None
    if per is not None:
        LAST_EXEC_NS = None
        O = np.concatenate(
            [per[k].reshape(14, PAD_PTS)[:, :PTS_PER_CORE] for k in range(N_CORES)],
            axis=1).astype(np.float32)  # [14, 65536]
    else:
        from concourse.bass_utils import run_bass_kernel_spmd
        kw = {}
        if trace:
            kw["trace"] = True
            td = os.environ.get("BASS_KERNEL_TRACE_DIR")
            if td:
                kw["tmpdir"] = td
        res = run_bass_kernel_spmd(nc, in_maps, list(range(N_CORES)), **kw)
        LAST_EXEC_NS = res.exec_time_ns
        O = np.concatenate(
            [np.asarray(res.results[k]["OUT"]).reshape(14, PAD_PTS)[:, :PTS_PER_CORE]
             for k in range(N_CORES)],
            axis=1).astype(np.float32)  # [14, 65536]

    lam1 = np.float32(np.asarray(inputs["lam1"]).reshape(-1)[0])
    lam2 = np.float32(np.asarray(inputs["lam2"]).reshape(-1)[0])
    u = O[1].copy()
    vv = (-O[0]).astype(np.float32)
    p = O[11].copy()
    f_u = O[6] + lam1 * (O[1] * O[3] - O[0] * O[4]) + O[12] - lam2 * (O[8] + O[10])
    f_v = -O[5] + lam1 * (O[0] * O[3] - O[1] * O[2]) + O[13] + lam2 * (O[7] + O[9])
    return (u, vv, p[:, None].copy(),
            f_u.astype(np.float32), f_v.astype(np.float32))


# revision 56
# speedup vs baseline: 1.8651x; 1.0309x over previous
import importlib.util
import os
import sys

sys.path.insert(0, "/opt/trn_rl_repo")

import numpy as np
from contextlib import ExitStack

N_CORES = 8
N_PTS = 65536
PTS_PER_CORE = N_PTS // N_CORES  # 8192
GROUPS = 6                        # unit-groups of 20 partitions (120/128 used)
GS = 20                           # partition stride per group
F = 512                           # max points per instruction (PSUM bank cap)
NT = 3                            # super-tiles (ragged last: 512+512+342)
FPS = (512, 512, 342)
JOFF = (0, 512, 1024)
PTS_PER_GROUP = sum(FPS)          # 1366
PAD_PTS = GROUPS * PTS_PER_GROUP  # 8196 (only 4 pad points)
CH_LIST = ["val", "zx", "zy", "zt", "zxx", "zxy", "zyy", "zxt", "zyt",
           "zxxx", "zxxy", "zxyy", "zyyy"]
CH_IDX = {c: i for i, c in enumerate(CH_LIST)}
BANK = {"val": "A", "zx": "B", "zy": "C", "zt": "D", "zxx": "E", "zxy": "F",
        "zyy": "G", "zxt": "H", "zyt": "A", "zxxx": "B", "zxxy": "C",
        "zxyy": "D", "zyyy": "E"}
ROUND1 = ["val", "zx", "zy", "zt", "zxx", "zxy", "zyy", "zxt"]
ROUND2 = ["zyt", "zxxx", "zxxy", "zxyy", "zyyy"]
PIECES_OF = {
    "val": ["v"], "zx": ["hx"], "zy": ["hy"], "zt": ["ht"],
    "zxx": ["mnxx"], "zxy": ["mnxy"], "zyy": ["myy", "nyy"],
    "zxt": ["mnxt"], "zyt": ["mnyt"],
    "zxxx": ["r1xxx", "r2xxx", "r3xxx"], "zxxy": ["r1xxy", "r2xxy", "r3xxy"],
    "zxyy": ["r1xyy", "r2xyy", "r3xyy"], "zyyy": ["r1yyy", "r2yyy", "r3yyy"],
}
# output row -> (final-layer piece tile, unit offset within group)
ROWS = [("hx", 0), ("hy", 0), ("mxx", 0), ("mxy", 0), ("myy", 0),
        ("mnxt", 0), ("mnyt", 0), ("r1xxx", 0), ("r1xxy", 0), ("r1xyy", 0),
        ("r1yyy", 0), ("v", 1), ("hx", 1), ("hy", 1)]
TILE_ORDER = ["hx", "hy", "mxx", "mxy", "myy", "mnxt", "mnyt",
              "r1xxx", "r1xxy", "r1xyy", "r1yyy", "v"]

LAST_EXEC_NS = None


def _build_program():
    import concourse.bass as bass
    import concourse.bacc as bacc
    import concourse.tile as tile
    import concourse.mybir as mybir

    f32 = mybir.dt.float32
    AF = mybir.ActivationFunctionType
    ALU = mybir.AluOpType

    nc = bacc.Bacc("TRN2", target_bir_lowering=False, num_devices=N_CORES)
    J1 = nc.declare_dram_parameter("J1", [GROUPS, 3, 13, PTS_PER_GROUP], f32, isOutput=False)
    WBD = nc.declare_dram_parameter("WBD", [128, 7 * 128], f32, isOutput=False)
    BB = nc.declare_dram_parameter("BB", [128, 7], f32, isOutput=False)
    SEL = nc.declare_dram_parameter("SEL", [128, 12 * 84], f32, isOutput=False)
    OUT = nc.declare_dram_parameter("OUT", [14, GROUPS, PTS_PER_GROUP], f32, isOutput=True)

    with ExitStack() as ctx:
        tc = ctx.enter_context(tile.TileContext(nc))
        const = ctx.enter_context(tc.tile_pool(name="const", bufs=1))
        jets = ctx.enter_context(tc.tile_pool(name="jets", bufs=1))
        pieces = ctx.enter_context(tc.tile_pool(name="pieces", bufs=2))
        work = ctx.enter_context(tc.tile_pool(name="work", bufs=1))
        psum = ctx.enter_context(tc.tile_pool(name="psum", bufs=1, space=bass.MemorySpace.PSUM))

        wbd = const.tile([128, 7 * 128], f32, name="wbd")
        bb = const.tile([128, 7], f32, name="bb")
        sel = const.tile([128, 12 * 84], f32, name="sel")
        # layer-2 weights + bias first so compute can start as soon as the
        # first super-tile's jets land; remaining constants load behind them
        nc.sync.dma_start(wbd[:, 0:128], WBD[:, 0:128])
        nc.sync.dma_start(bb[:], BB[:])

        ps = {k: psum.tile([128, F], f32, name=f"ps{k}") for k in "ABCDEFGH"}

        def pt(name):
            return pieces.tile([128, F], f32, name=name)

        def wt(name):
            return work.tile([128, F], f32, name=name)

        def emit_round(l, chs, prev, ji, fp):
            off = 128 * (l - 2)
            lhsT = wbd[:, off:off + 128]
            for chn in chs:
                bank = ps[BANK[chn]]
                if l == 2:
                    srcs = [ji[:, CH_IDX[chn] * fp:(CH_IDX[chn] + 1) * fp]]
                else:
                    srcs = [prev[p][:, :fp] for p in PIECES_OF[chn]]
                for i, src in enumerate(srcs):
                    nc.tensor.matmul(bank[:, :fp], lhsT, src,
                                     start=(i == 0), stop=(i == len(srcs) - 1))

        def emit_act(l, fp):
            v = pt("v")
            nc.scalar.activation(v[:, :fp], ps["A"][:, :fp], AF.Tanh, bias=bb[:, l - 2:l - 1])
            sq = wt("sq")
            nc.scalar.activation(sq[:, :fp], v[:, :fp], AF.Square)
            f1 = pt("f1")
            nc.scalar.activation(f1[:, :fp], sq[:, :fp], AF.Copy, bias=1.0, scale=-1.0)
            c = {}
            for nm, bk in [("x", "B"), ("y", "C"), ("t", "D"),
                           ("xx", "E"), ("xy", "F"), ("yy", "G")]:
                cc = wt("c" + nm)
                nc.scalar.activation(cc[:, :fp], ps[bk][:, :fp], AF.Copy)
                c[nm] = cc
            return v, sq, f1, c

        def emit_dve(l, v, sq, f1, c, last, fp):
            def S(x):
                return x[:, :fp]
            P = {}
            # free PSUM banks as early as possible
            nxt = wt("nxt"); nc.vector.tensor_mul(S(nxt), S(f1), S(ps["H"]))
            nyt = wt("nyt"); nc.vector.tensor_mul(S(nyt), S(f1), S(ps["A"]))
            r3 = {}
            for abc, bk in [("xxx", "B"), ("xxy", "C"), ("xyy", "D"), ("yyy", "E")]:
                r = pt("r3" + abc); nc.vector.tensor_mul(S(r), S(f1), S(ps[bk]))
                r3[abc] = r
            f2h = wt("f2h")
            nc.vector.scalar_tensor_tensor(S(f2h), S(sq), 1.0, S(v), ALU.subtract, ALU.mult)
            f3g = wt("f3g")
            nc.vector.scalar_tensor_tensor(S(f3g), S(sq), 1.0 / 3.0, S(f1), ALU.subtract, ALU.mult)
            hx = pt("hx"); nc.vector.tensor_mul(S(hx), S(f1), S(c["x"]))
            hy = pt("hy"); nc.vector.tensor_mul(S(hy), S(f1), S(c["y"]))
            ht = None
            if not last:
                ht = pt("ht"); nc.vector.tensor_mul(S(ht), S(f1), S(c["t"]))
            for ab, (a, b) in [("xx", ("x", "x")), ("xy", ("x", "y")), ("yy", ("y", "y")),
                               ("xt", ("x", "t")), ("yt", ("y", "t"))]:
                pp = wt("p" + ab); nc.gpsimd.tensor_mul(S(pp), S(c[a]), S(c[b]))
                P[ab] = pp
            m = {}
            for ab in ["xx", "xy", "yy", "xt", "yt"]:
                if ab in ("xx", "xy", "yy"):
                    mm = pt("m" + ab)
                elif last:
                    # must outlive this tile (read by deferred compaction)
                    mm = pt("mn" + ab)
                else:
                    mm = wt("m" + ab)
                nc.vector.scalar_tensor_tensor(S(mm), S(P[ab]), 2.0, S(f2h), ALU.mult, ALU.mult)
                m[ab] = mm
            n = {"xt": nxt, "yt": nyt}
            for ab, eng in [("xx", nc.gpsimd), ("xy", nc.gpsimd), ("yy", nc.vector)]:
                nn = pt("n" + ab); eng.tensor_mul(S(nn), S(f1), S(c[ab]))
                n[ab] = nn
            if not last:
                # pre-fold xx/xy on DVE so the next layer needs 1 matmul each
                for ab in ["xx", "xy"]:
                    nc.vector.tensor_add(S(m[ab]), S(m[ab]), S(n[ab]))
            q = {}
            for qi, (a, b) in [("1", ("xx", "x")), ("2", ("xx", "y")), ("3", ("xy", "x")),
                               ("4", ("xy", "y")), ("5", ("yy", "x")), ("6", ("yy", "y"))]:
                qq = wt("q" + qi); nc.gpsimd.tensor_mul(S(qq), S(c[a]), S(c[b]))
                q[qi] = qq
            sxxy = wt("sxxy")
            nc.vector.scalar_tensor_tensor(S(sxxy), S(q["3"]), 2.0, S(q["2"]), ALU.mult, ALU.add)
            sxyy = wt("sxyy")
            nc.vector.scalar_tensor_tensor(S(sxyy), S(q["4"]), 2.0, S(q["5"]), ALU.mult, ALU.add)
            T = {}
            for abc, (pab, a) in [("xxx", ("xx", "x")), ("xxy", ("xx", "y")),
                                  ("xyy", ("yy", "x")), ("yyy", ("yy", "y"))]:
                tt = wt("t" + abc); nc.gpsimd.tensor_mul(S(tt), S(P[pab]), S(c[a]))
                T[abc] = tt
            r1 = {}
            for abc in ["xxx", "xxy", "xyy", "yyy"]:
                rr = pt("r1" + abc)
                nc.vector.scalar_tensor_tensor(S(rr), S(T[abc]), 6.0, S(f3g), ALU.mult, ALU.mult)
                r1[abc] = rr
            r2 = {}
            for abc, (src, k) in [("xxx", (q["1"], 6.0)), ("xxy", (sxxy, 2.0)),
                                  ("xyy", (sxyy, 2.0)), ("yyy", (q["6"], 6.0))]:
                rr = pt("r2" + abc)
                nc.vector.scalar_tensor_tensor(S(rr), S(src), k, S(f2h), ALU.mult, ALU.mult)
                r2[abc] = rr

            if not last:
                out = {"v": v, "hx": hx, "hy": hy, "ht": ht}
                for ab in ["xt", "yt"]:
                    z = pt("mn" + ab)
                    nc.vector.tensor_add(S(z), S(m[ab]), S(n[ab]))
                    out["mn" + ab] = z
                out["mnxx"] = m["xx"]
                out["mnxy"] = m["xy"]
                out["myy"] = m["yy"]
                out["nyy"] = n["yy"]
                for abc in ["xxx", "xxy", "xyy", "yyy"]:
                    out["r1" + abc] = r1[abc]
                    out["r2" + abc] = r2[abc]
                    out["r3" + abc] = r3[abc]
                return out
            # last layer: fold pieces into final jets (in-place adds)
            for ab in ["xx", "xy", "yy", "xt", "yt"]:
                nc.vector.tensor_add(S(m[ab]), S(m[ab]), S(n[ab]))
            for abc in ["xxx", "xxy", "xyy", "yyy"]:
                nc.vector.tensor_add(S(r1[abc]), S(r1[abc]), S(r2[abc]))
                nc.vector.tensor_add(S(r1[abc]), S(r1[abc]), S(r3[abc]))
            return {"hx": hx, "hy": hy, "mxx": m["xx"], "mxy": m["xy"],
                    "myy": m["yy"], "mnxt": m["xt"], "mnyt": m["yt"],
                    "r1xxx": r1["xxx"], "r1xxy": r1["xxy"], "r1xyy": r1["xyy"],
                    "r1yyy": r1["yyy"], "v": v}

        ji = jets.tile([128, 13 * F], f32, name="ji")
        nc.vector.memset(ji[:], 0.0)

        def load_ji(t):
            # per-tile packed layout: channel ch occupies ji cols [ch*fp, (ch+1)*fp)
            fp = FPS[t]
            for g in range(GROUPS):
                nc.sync.dma_start(ji[GS * g:GS * g + 3, 0:13 * fp],
                                  J1[g, :, :, JOFF[t]:JOFF[t] + fp])

        def emit_compact(fin, t, fp):
            # gather the 14 output rows x 6 groups into partitions 6*row+g of
            # one PSUM bank via selection matmuls, then 1 copy + 1 DMA descriptor
            bank = ps["H"]
            for i, nm in enumerate(TILE_ORDER):
                nc.tensor.matmul(bank[0:84, :fp], sel[:, 84 * i:84 * (i + 1)],
                                 fin[nm][:, :fp], start=(i == 0), stop=(i == 11))
            cons = wt("cons")
            nc.scalar.activation(cons[0:84, :fp], bank[0:84, :fp], AF.Copy)
            nc.sync.dma_start(OUT[:, :, JOFF[t]:JOFF[t] + fp], cons[0:84, :fp])

        load_ji(0)
        # non-critical constants load behind the first jets
        nc.sync.dma_start(wbd[:, 128:7 * 128], WBD[:, 128:7 * 128])
        nc.sync.dma_start(sel[:], SEL[:])
        fin_prev = None
        for t in range(NT):
            fp = FPS[t]
            prev = None
            for l in range(2, 9):
                if l == 3 and fin_prev is not None:
                    # previous tile's output consolidation, overlapped here
                    emit_compact(*fin_prev)
                    fin_prev = None
                emit_round(l, ROUND1, prev, ji, fp)
                v, sq, f1, c = emit_act(l, fp)
                emit_round(l, ROUND2, prev, ji, fp)
                if l == 2 and t + 1 < NT:
                    load_ji(t + 1)
                prev = emit_dve(l, v, sq, f1, c, last=(l == 8), fp=fp)
            fin_prev = (prev, t, fp)
        emit_compact(*fin_prev)

    nc.finalize()
    return nc


_NC = None


def _get_nc():
    global _NC
    if _NC is None:
        _NC = _build_program()
    return _NC


def _host_pack(inputs):
    X32 = np.asarray(inputs["X"], dtype=np.float32)
    X = X32.astype(np.float64)
    Ws = [np.asarray(inputs[f"W{i}"], dtype=np.float64) for i in range(1, 9)]
    bs = [np.asarray(inputs[f"b{i}"], dtype=np.float64) for i in range(1, 9)]

    lb = float(X32[:, 0].min())
    ub = float(X32[:, 0].max())
    s = 2.0 / (ub - lb)
    cshift = -2.0 * lb / (ub - lb) - 1.0
    W1e = s * Ws[0]                      # [3,3]
    b1e = bs[0] + cshift * Ws[0].sum(axis=0)

    Z1 = X @ W1e + b1e                   # [N,3]
    y = np.tanh(Z1)
    sq = y * y
    f1 = 1.0 - sq
    f2 = -2.0 * y * f1
    f3 = f1 * (6.0 * sq - 2.0)
    ux, uy, ut = W1e[0], W1e[1], W1e[2]  # each [3]

    jets = np.empty((13, N_PTS, 3), dtype=np.float64)
    jets[0] = y
    jets[1] = f1 * ux
    jets[2] = f1 * uy
    jets[3] = f1 * ut
    jets[4] = f2 * (ux * ux)
    jets[5] = f2 * (ux * uy)
    jets[6] = f2 * (uy * uy)
    jets[7] = f2 * (ux * ut)
    jets[8] = f2 * (uy * ut)
    jets[9] = f3 * (ux * ux * ux)
    jets[10] = f3 * (ux * ux * uy)
    jets[11] = f3 * (ux * uy * uy)
    jets[12] = f3 * (uy * uy * uy)
    jets = jets.astype(np.float32)

    WBD = np.zeros((128, 7 * 128), dtype=np.float32)
    BBp = np.zeros((128, 7), dtype=np.float32)
    for l in range(2, 9):
        W = Ws[l - 1].astype(np.float32)
        b = bs[l - 1].astype(np.float32)
        din, dout = W.shape
        off = 128 * (l - 2)
        for g in range(GROUPS):
            WBD[GS * g:GS * g + din, off + GS * g:off + GS * g + dout] = W
            BBp[GS * g:GS * g + dout, l - 2] = b

    SELp = np.zeros((128, 12 * 84), dtype=np.float32)
    for i, nm in enumerate(TILE_ORDER):
        for r, (rnm, unit) in enumerate(ROWS):
            if rnm != nm:
                continue
            for g in range(GROUPS):
                SELp[GS * g + unit, 84 * i + 6 * r + g] = 1.0

    in_maps = []
    for k in range(N_CORES):
        A = jets[:, PTS_PER_CORE * k:PTS_PER_CORE * (k + 1), :]      # [13,8192,3]
        Ap = np.zeros((13, PAD_PTS, 3), dtype=np.float32)
        Ap[:, :PTS_PER_CORE] = A
        B = Ap.reshape(13, GROUPS, PTS_PER_GROUP, 3)
        J1k = np.ascontiguousarray(B.transpose(1, 3, 0, 2))  # [G,3,13,1366]
        in_maps.append({"J1": J1k, "WBD": WBD, "BB": BBp, "SEL": SELp})
    return in_maps


_RUN = None
_RUN_BROKEN = False


def _get_runner():
    # cache the jitted executable so repeat kernel() calls skip re-trace,
    # re-lowering and NEFF recompilation (which dominate wall time)
    global _RUN
    if _RUN is not None:
        return _RUN
    import jax
    from concourse import bass2jax, mybir

    nc = _get_nc()
    bass2jax.install_neuronx_cc_hook()
    partition_name = nc.partition_id_tensor.name if nc.partition_id_tensor else None
    in_names, out_names, out_avals, zero_shapes = [], [], [], []
    for alloc in nc.m.functions[0].allocations:
        if not isinstance(alloc, mybir.MemoryLocationSet):
            continue
        name = alloc.memorylocations[0].name
        if alloc.kind == "ExternalInput":
            if name != partition_name:
                in_names.append(name)
        elif alloc.kind == "ExternalOutput":
            shape = tuple(alloc.tensor_shape)
            dtype = mybir.dt.np(alloc.dtype)
            out_names.append(name)
            out_avals.append(jax.core.ShapedArray(shape, dtype))
            zero_shapes.append((shape, dtype))
    n_params = len(in_names)
    n_outs = len(out_names)
    all_in = list(in_names) + list(out_names)
    if partition_name is not None:
        all_in.append(partition_name)
    donate = tuple(range(n_params, n_params + n_outs))

    def _body(*args):
        operands = list(args)
        if partition_name is not None:
            operands.append(bass2jax.partition_id_tensor())
        return tuple(bass2jax._bass_exec_p.bind(
            *operands,
            out_avals=tuple(out_avals),
            in_names=tuple(all_in),
            out_names=tuple(out_names),
            lowering_input_output_aliases=(),
            sim_require_finite=True,
            sim_require_nnan=True,
            nc=nc,
        ))

    devices = jax.devices()[:N_CORES]
    mesh = bass2jax.Mesh(np.asarray(devices), ("core",))
    in_specs = (bass2jax.PartitionSpec("core"),) * (n_params + n_outs)
    out_specs = (bass2jax.PartitionSpec("core"),) * n_outs
    sharded = jax.jit(
        bass2jax.shard_map(_body, mesh=mesh, in_specs=in_specs,
                           out_specs=out_specs, check_rep=False),
        donate_argnums=donate, keep_unused=True)
    _RUN = (sharded, in_names, out_avals, zero_shapes, n_params)
    return _RUN


def _run_cached(in_maps):
    sharded, in_names, out_avals, zero_shapes, n_params = _get_runner()
    concat_in = [
        np.concatenate([np.asarray(in_maps[c][nm]) for c in range(N_CORES)], axis=0)
        for nm in in_names]
    concat_zeros = [
        np.zeros((N_CORES * shape[0], *shape[1:]), dtype)
        for (shape, dtype) in zero_shapes]
    out_arrs = sharded(*concat_in, *concat_zeros)
    per = np.asarray(out_arrs[0]).reshape(N_CORES, *out_avals[0].shape)
    return per


def kernel(**inputs):
    global LAST_EXEC_NS, _RUN_BROKEN
    nc = _get_nc()
    in_maps = _host_pack(inputs)
    trace = bool(os.environ.get("BASS_KERNEL_TRACE"))
    if trace and importlib.util.find_spec("antenv.axon_hooks") is None:
        trace = False
    per = None
    if not trace and not _RUN_BROKEN:
        try:
            per = _run_cached(in_maps)
        except Exception:
            _RUN_BROKEN = True
            per = None
    if per is not None:
        LAST_EXEC_NS = None
        O = np.concatenate(
            [per[k].reshape(14, PAD_PTS)[:, :PTS_PER_CORE] for k in range(N_CORES)],
            axis=1).astype(np.float32)  # [14, 65536]
    else:
        from concourse.bass_utils import run_bass_kernel_spmd
        kw = {}
        if trace:
            kw["trace"] = True
            td = os.environ.get("BASS_KERNEL_TRACE_DIR")
            if td:
                kw["tmpdir"] = td
        res = run_bass_kernel_spmd(nc, in_maps, list(range(N_CORES)), **kw)
        LAST_EXEC_NS = res.exec_time_ns
        O = np.concatenate(
            [np.asarray(res.results[k]["OUT"]).reshape(14, PAD_PTS)[:, :PTS_PER_CORE]
             for k in range(N_CORES)],
            axis=1).astype(np.float32)  # [14, 65536]

    lam1 = np.float32(np.asarray(inputs["lam1"]).reshape(-1)[0])
    lam2 = np.float32(np.asarray(inputs["lam2"]).reshape(-1)[0])
    u = O[1].copy()
    vv = (-O[0]).astype(np.float32)
    p = O[11].copy()
    f_u = O[6] + lam1 * (O[1] * O[3] - O[0] * O[4]) + O[12] - lam2 * (O[8] + O[10])
    f_v = -O[5] + lam1 * (O[0] * O[3] - O[1] * O[2]) + O[13] + lam2 * (O[7] + O[9])
    return (u, vv, p[:, None].copy(),
            f_u.astype(np.float32), f_v.astype(np.float32))
